# revision 67
# baseline (speedup 1.0000x reference)
"""Trainium2 Bass kernel for masked causal attention with RoPE (mgdt column masking).

Reference computation (B=4, T=2048, H=512, heads=8, D=64):
  q/k/v = x @ W + b;  RoPE(q, k) over full hidden dim (pairs of adjacent channels);
  scores = q k^T / sqrt(D) with causal tril mask plus fully-masked key columns
  at {4, 7, 10, ...} (period 3); softmax; out = (att @ v) @ Wo + bo.

Sharding: 8 cores = data-parallel over batch (4) x tensor-parallel over head
groups (2 x 4 heads). Each core computes a [T, H] partial of its batch's
output projection (Wo row-sharded); host sums the pair of partials + bo.

Key structural choices (v2):
  - KEY COMPACTION: the 682 fully-masked key columns are removed on the host
    (x^T gathered at the 1366 unmasked positions, padded to 1408).
  - q/k are computed TRANSPOSED as qT[c, t] (c on partitions) in FP8 (e4m3)
    with DoubleRow matmuls (2 K-subtiles of 128 per instruction -> half the
    PE passes of bf16). Softmax washes out the fp8 noise on the scores path;
    V stays bf16 (its quantization error would land directly on the output).
  - RoPE without the swapped second projection: qrot = (q+b)*C + P@u,
    u = (q+b)*S~ where S~ is the sin table pre-swapped/sign-folded on the
    host and P is a 128x128 pair-swap permutation matmul (engines cannot
    partition-step; one K=128 F=512 matmul replaces two DR passes).
    bv is folded through Wo on the host (bo2 = bo + bv @ Wo), so the V
    projection has no bias pass; the denominator ones-column is restored
    with a tiny broadcast copy of per-tile valid flags.
  - scores are computed transposed (sT[s, t] per head, K=64 contraction, two
    heads packed in one [128, .] PSUM via PE row-tiling) so softmax's
    s-reduction and att@v's s-contraction keep s on partitions.
  - p = exp(sT/8) with no max subtraction (|scores/8| < ~2 by construction).
  - V is augmented per head with a ones column -> att@v emits the softmax
    denominator as row 64 of its PSUM output for free.
  - Normalization: denominators gathered (DMA), reciprocal'd in one [128, 8]
    DVE op, scattered back, broadcast over partitions with a K=2 selector
    matmul, applied with ONE [128, 512] multiply per (chunk, head-pair).
  - Inputs are loaded in consumption order, chunked, split across both HW
    DMA queues (sync + scalar) so the PE starts within ~1us. Trig tables and
    band masks ship as fp8; the output partial ships as bf16.
"""

import sys

if "/opt/trn_rl_repo" not in sys.path:
    sys.path.insert(0, "/opt/trn_rl_repo")

import numpy as np
import ml_dtypes

B, T, H, NH, D = 4, 2048, 512, 8, 64
THETA = 10000.0
PERIOD, RET_ORDER = 3, 2
NCORES = 8
CPG = H // 2          # 256 channels per head-group shard
CHUNK = 512           # t-chunk (one PSUM bank of fp32)
NCH = T // CHUNK      # 4 query chunks
BF = ml_dtypes.bfloat16
F8 = ml_dtypes.float8_e4m3

# --- compacted key geometry (host + builder share this) ---
_cm = np.ones(T, bool)
_cm[PERIOD + RET_ORDER - 1::PERIOD] = False
POS = np.where(_cm)[0]              # 1366 unmasked key positions
NSC_RAW = len(POS)                  # 1366
NSTC = (NSC_RAW + 127) // 128       # 11 s-tiles
NSC = NSTC * 128                    # 1408 padded
KCW = [CHUNK, CHUNK, NSC - 2 * CHUNK]   # k-projection chunk widths (512,512,384)

# per s-tile first/last valid original positions
_INF = 1 << 30
TILE_LO = [int(POS[128 * i]) if 128 * i < NSC_RAW else _INF for i in range(NSTC)]
TILE_HI = [int(POS[min(128 * i + 127, NSC_RAW - 1)]) if 128 * i < NSC_RAW else _INF
           for i in range(NSTC)]


def _tiles_for_chunk(j):
    """(i, col0, crossing) for each compact s-tile contributing to t-chunk j."""
    out = []
    for i in range(NSTC):
        lo, hi = TILE_LO[i], TILE_HI[i]
        if lo > CHUNK * j + CHUNK - 1:
            continue
        col0 = max(0, lo - CHUNK * j)
        crossing = hi > CHUNK * j  # some (row, col) pairs invalid -> needs mask
        out.append((i, col0, crossing))
    return out


_CROSSINGS = sorted({(i, j) for j in range(NCH)
                     for (i, c0, cr) in _tiles_for_chunk(j) if cr})

_prog = None


def _build_program():
    global _prog
    if _prog is not None:
        return _prog
    from contextlib import ExitStack
    import concourse.bacc as bacc
    import concourse.tile as tile
    import concourse.bass as _bass
    from concourse import mybir

    bf = mybir.dt.bfloat16
    f8 = mybir.dt.float8e4
    f32 = mybir.dt.float32
    EXP = mybir.ActivationFunctionType.Exp
    ADD = mybir.AluOpType.add
    MULT = mybir.AluOpType.mult
    DR = mybir.MatmulPerfMode.DoubleRow

    nc = bacc.Bacc("TRN2", target_bir_lowering=False, debug=False, num_devices=NCORES)

    def din(name, shape, dt):
        return nc.dram_tensor(name, shape, dt, kind="ExternalInput").ap()

    nm = len(_CROSSINGS)
    xt8_d = din("xt8", [128, 4 * T], f8)           # x^T 4 row-tiles side by side
    xtc8_d = din("xtc8", [128, 4 * NSC], f8)       # compacted x^T row-tiles
    xtcv_d = din("xtcv", [128, 4 * NSC], bf)       # bf16 copy for V projection
    wqk8_d = din("wqk8", [128, 2048], f8)          # q|k x ct x pass, 256-col blocks
    wv_d = din("wv", [128, 4 * 260], bf)
    wo_d = din("wo", [128, 2 * H], bf)
    # trig layout: [Cq ct0 | Sq~ ct0 | Cq ct1 | Sq~ ct1 | Ck ct0 | Sk~ ct0 |
    #               Ck ct1 | Sk~ ct1] so each ct's cos+sin are adjacent
    trig_d = din("trig", [128, 2 * (2 * T + 2 * NSC)], f8)
    bm_d = din("bmask", [128, nm * CHUNK], f8)
    sel_d = din("sel", [2, 128], bf)
    perm_d = din("perm", [128, 128], bf)           # partition pair-swap matrix
    vone_d = din("vones", [128, NSTC], f32)        # per s-tile valid-row flags
    bias_d = din("biases", [128, 4], f32)          # bq ct0|bq ct1|bk ct0|bk ct1
    out_d = nc.dram_tensor("out", [T, H], bf, kind="ExternalOutput").ap()

    TRIG_K = 4 * T

    with tile.TileContext(nc) as tc:
        with ExitStack() as ctx:
            sg = ctx.enter_context(tc.tile_pool(name="sg", bufs=1))

            def ld(name, dram, cols, dt=bf, nsplit=4, eng=None):
                eng = eng or nc.sync
                tl = sg.tile([128, cols], dt, tag=name, name=name)
                step = -(-cols // nsplit)
                for a in range(0, cols, step):
                    b = min(a + step, cols)
                    eng.dma_start(out=tl[:, a:b], in_=dram[:, a:b])
                return tl

            # ---- input loads in consumption order, split across 2 HW queues
            # sync queue: only the big projection inputs, first-use first;
            # fewer chunks = fewer 0.6us issue slots + less DMA-sem recycling
            wqk8 = sg.tile([128, 2048], f8, tag="wqk8", name="wqk8")
            nc.sync.dma_start(out=wqk8[:, 1024:2048], in_=wqk8_d[:, 1024:2048])
            xtc8 = ld("xtc8", xtc8_d, 4 * NSC, dt=f8, nsplit=2)
            trigk = sg.tile([128, 4 * NSC], f8, tag="trigk", name="trigk")
            for a in range(0, 4 * NSC, 2 * NSC):
                nc.sync.dma_start(out=trigk[:, a:a + 2 * NSC],
                                  in_=trig_d[:, TRIG_K + a:TRIG_K + a + 2 * NSC])
            nc.sync.dma_start(out=wqk8[:, 0:1024], in_=wqk8_d[:, 0:1024])
            xt8 = ld("xt8", xt8_d, 4 * T, dt=f8, nsplit=2)
            trigq = sg.tile([128, 4 * T], f8, tag="trigq", name="trigq")
            for a in range(0, 4 * T, 2 * T):
                nc.sync.dma_start(out=trigq[:, a:a + 2 * T],
                                  in_=trig_d[:, a:a + 2 * T])
            # scalar queue: small constants, then v projection + attention inputs
            bias_sb = sg.tile([128, 4], f32, tag="biases")
            nc.scalar.dma_start(out=bias_sb, in_=bias_d[:, :])
            perm_sb = sg.tile([128, 128], bf, tag="perm")
            nc.scalar.dma_start(out=perm_sb, in_=perm_d[:, :])
            sel_sb = sg.tile([2, 128], bf, tag="sel")
            nc.scalar.dma_start(out=sel_sb, in_=sel_d[:, :])
            vone_sb = sg.tile([128, NSTC], f32, tag="vones")
            nc.scalar.dma_start(out=vone_sb, in_=vone_d[:, :])
            xtcv = ld("xtcv", xtcv_d, 4 * NSC, dt=bf, nsplit=2, eng=nc.scalar)
            wvall = ld("wvall", wv_d, 4 * 260, dt=bf, nsplit=1, eng=nc.scalar)
            bmall = ld("bmall", bm_d, nm * CHUNK, dt=f8, nsplit=1, eng=nc.scalar)
            woall = ld("woall", wo_d, 2 * H, dt=bf, nsplit=1, eng=nc.scalar)

            def wqk(v, ct, p):
                """stationary [128, 2, 128] for variant v (0=q, 1=k), ct, pass p."""
                base = 1024 * v + 512 * ct + 256 * p
                t = wqk8[:, base:base + 256]
                return _bass.AP(tensor=t.tensor, offset=t.offset,
                                ap=[t.ap[0], [128, 2], [1, 128]])

            def dr_rhs(xtile, tile_cols, p, csl):
                """moving [128, 2, w] = row-tile pair (2p, 2p+1), cols csl."""
                t = xtile[:, tile_cols * 2 * p + csl.start:
                          tile_cols * 2 * p + csl.stop]
                return _bass.AP(tensor=t.tensor, offset=t.offset,
                                ap=[t.ap[0], [tile_cols, 2],
                                    [1, csl.stop - csl.start]])

            def bm_slice(i, j, c0):
                n = _CROSSINGS.index((i, j))
                return bmall[:, CHUNK * n + c0:CHUNK * (n + 1)]

            # persistent activations
            qrot, krot, aot = {}, {}, {}
            for ct in range(2):
                for ch in range(NCH):
                    qrot[ct, ch] = sg.tile([128, CHUNK], bf, tag=f"qr{ct}_{ch}",
                                           name=f"qr{ct}_{ch}")
                    aot[ct, ch] = sg.tile([128, CHUNK], bf, tag=f"ao{ct}_{ch}",
                                          name=f"ao{ct}_{ch}")
                for kc in range(3):
                    krot[ct, kc] = sg.tile([128, CHUNK], bf, tag=f"kr{ct}_{kc}",
                                           name=f"kr{ct}_{kc}")
            vaug = []
            for s in range(NSTC):
                vaug.append(sg.tile([128, 260], bf, tag=f"va{s}", name=f"va{s}"))

            pp = ctx.enter_context(tc.tile_pool(name="pp", bufs=8))
            rtmp = ctx.enter_context(tc.tile_pool(name="rtmp", bufs=4))
            dn = ctx.enter_context(tc.tile_pool(name="dn", bufs=4))
            stg = ctx.enter_context(tc.tile_pool(name="stg", bufs=4))
            ost = ctx.enter_context(tc.tile_pool(name="ost", bufs=4))

            # ---- phase B: K/V/Q projections + rope ----
            with tc.tile_pool(name="ppj", bufs=4, space="PSUM") as ppj, \
                 tc.tile_pool(name="ppw", bufs=2, space="PSUM") as ppw, \
                 tc.tile_pool(name="ppv", bufs=2, space="PSUM") as ppv:

                def proj_stage1(ct, var, bcol, dst, xtile, tile_cols, csl,
                                csb, ssb, tag):
                    """pm = x8 @ W8 (fp8 DoubleRow); dst = (pm+b)*C; u = (pm+b)*S~."""
                    w = csl.stop - csl.start
                    ps = ppj.tile([128, CHUNK], f32, tag="ps", name=f"pj_{tag}")
                    pm = ps[:, 0:w]
                    for p in range(2):
                        nc.tensor.matmul(pm, lhsT=wqk(var, ct, p),
                                         rhs=dr_rhs(xtile, tile_cols, p, csl),
                                         start=(p == 0), stop=(p == 1),
                                         perf_mode=DR)
                    nc.vector.scalar_tensor_tensor(
                        out=dst[:, :w], in0=pm, scalar=bias_sb[:, bcol:bcol + 1],
                        in1=csb, op0=ADD, op1=MULT)
                    u = rtmp.tile([128, CHUNK], bf, tag="u")
                    nc.vector.scalar_tensor_tensor(
                        out=u[:, :w], in0=pm, scalar=bias_sb[:, bcol:bcol + 1],
                        in1=ssb, op0=ADD, op1=MULT)
                    return dst, u, w, tag

                def proj_stage2(dst, u, w, tag):
                    """dst += P @ u (partition-pair swap via perm matmul)."""
                    psw = ppw.tile([128, CHUNK], f32, tag="psw",
                                   name=f"psw_{tag}")
                    nc.tensor.matmul(psw[:, 0:w], lhsT=perm_sb, rhs=u[:, :w],
                                     start=True, stop=True)
                    nc.vector.tensor_add(dst[:, :w], dst[:, :w], psw[:, 0:w])

                pend = []
                for ct in range(2):
                    for kc, w in enumerate(KCW):
                        csl = slice(CHUNK * kc, CHUNK * kc + w)
                        if len(pend) >= 2:
                            proj_stage2(*pend.pop(0))
                        pend.append(proj_stage1(
                            ct, 1, 2 + ct, krot[ct, kc], xtc8, NSC, csl,
                            trigk[:, 2 * NSC * ct + csl.start:
                                  2 * NSC * ct + csl.stop],
                            trigk[:, 2 * NSC * ct + NSC + csl.start:
                                  2 * NSC * ct + NSC + csl.stop],
                            f"k{ct}_{kc}"))
                for s in range(NSTC):
                    ssl = slice(128 * s, 128 * (s + 1))
                    pv = ppv.tile([128, 260], f32, tag="pv", name=f"pv{s}")
                    for kt in range(4):
                        nc.tensor.matmul(pv, lhsT=xtcv[:, NSC * kt + ssl.start:
                                                       NSC * kt + ssl.stop],
                                         rhs=wvall[:, 260 * kt:260 * (kt + 1)],
                                         start=(kt == 0), stop=(kt == 3))
                    nc.scalar.copy(out=vaug[s], in_=pv)
                    # denominator ones-column (0 at pad rows), bv is folded
                    # into the host-side bias via bv @ Wo
                    vo = vone_sb[:, s:s + 1]
                    vob = _bass.AP(tensor=vo.tensor, offset=vo.offset,
                                   ap=[vo.ap[0], [0, 4]])
                    nc.vector.tensor_copy(out=vaug[s][:, 64:260:65], in_=vob)
                    if pend and s % 2 == 1:
                        proj_stage2(*pend.pop(0))
                for j in range(NCH):
                    for ct in range(2):
                        csl = slice(CHUNK * j, CHUNK * (j + 1))
                        if len(pend) >= 2:
                            proj_stage2(*pend.pop(0))
                        pend.append(proj_stage1(
                            ct, 0, ct, qrot[ct, j], xt8, T, csl,
                            trigq[:, 2 * T * ct + csl.start:
                                  2 * T * ct + csl.stop],
                            trigq[:, 2 * T * ct + T + csl.start:
                                  2 * T * ct + T + csl.stop],
                            f"q{ct}_{j}"))
                while pend:
                    proj_stage2(*pend.pop(0))

            pps = ctx.enter_context(
                tc.tile_pool(name="pps", bufs=2, space="PSUM"))
            ppo = ctx.enter_context(tc.tile_pool(name="ppo", bufs=3, space="PSUM"))
            pprd = ctx.enter_context(tc.tile_pool(name="pprd", bufs=1, space="PSUM"))

            # ---- attention + output projection ----
            def atth(j):
                """scores/exp/mask/attv for both head pairs of chunk j, plus
                PSUM->SBUF staging and the reciprocal-denominator path.
                Returns per-hp staging handles for finish(j)."""
                tiles_j = _tiles_for_chunk(j)
                ret = []
                for hp in range(2):
                    ct = hp
                    po = [ppo.tile([65, CHUNK], f32, tag="po",
                                   name=f"po{j}_{hp}_{i}") for i in range(2)]
                    for si, (s, col0, crossing) in enumerate(tiles_j):
                        first = si == 0
                        last = si == len(tiles_j) - 1
                        ksl = slice(128 * (s % 4), 128 * (s % 4) + 128)
                        ps = pps.tile([128, 2 * CHUNK], f32, tag="ps",
                                      name=f"ps{j}_{hp}_{s}")
                        for idx in range(2):
                            pb = 64 * idx
                            nc.tensor.matmul(
                                ps[:, CHUNK * idx + col0:CHUNK * (idx + 1)],
                                lhsT=krot[ct, s // 4][pb:pb + 64, ksl],
                                rhs=qrot[ct, j][pb:pb + 64, col0:],
                                start=True, stop=True)
                        pt = pp.tile([128, 2 * CHUNK], bf, tag="p",
                                     name=f"pt{j}_{hp}_{s}")
                        # per-head exp/mask/attv so attv(head0) overlaps
                        # exp(head1) instead of waiting the full pair
                        for idx in range(2):
                            csl2 = slice(CHUNK * idx + col0, CHUNK * (idx + 1))
                            nc.scalar.activation(out=pt[:, csl2], in_=ps[:, csl2],
                                                 func=EXP, scale=0.125)
                            if crossing:
                                bmb = bm_slice(s, j, col0)
                                nc.vector.tensor_mul(pt[:, csl2], pt[:, csl2],
                                                     bmb)
                            hh = 2 * hp + idx
                            nc.tensor.matmul(
                                po[idx][:, col0:],
                                lhsT=vaug[s][:, 65 * hh:65 * hh + 65],
                                rhs=pt[:, csl2],
                                start=first, stop=last,
                                skip_group_check=True)
                    # move PSUM results to SBUF (DMA cannot read PSUM)
                    oA = stg.tile([65, CHUNK], f32, tag="oA")
                    oB = stg.tile([65, CHUNK], f32, tag="oB")
                    nc.vector.tensor_copy(out=oA, in_=po[0])
                    nc.vector.tensor_copy(out=oB, in_=po[1])
                    shb = stg.tile([128, CHUNK], f32, tag="shb")
                    nc.sync.dma_start(out=shb[64:128, :], in_=oB[0:64, :])
                    dsb = dn.tile([128, 8], f32, tag="den")
                    nc.sync.dma_start(out=dsb[:, 0:4], in_=oA[64:65, :])
                    nc.sync.dma_start(out=dsb[:, 4:8], in_=oB[64:65, :])
                    rsb = dn.tile([128, 8], bf, tag="rden")
                    with nc.allow_low_precision(reason="bf16 softmax recip"):
                        nc.vector.reciprocal(rsb, dsb)
                    rdr = dn.tile([2, CHUNK], bf, tag="rdr")
                    nc.sync.dma_start(out=rdr[0:1, :], in_=rsb[:, 0:4])
                    nc.sync.dma_start(out=rdr[1:2, :], in_=rsb[:, 4:8])
                    ret.append((oA, shb, rdr))
                return ret

            def finish(j, data):
                """normalize chunk j (deferred past atth(j+1)) + out-projection."""
                for hp, (oA, shb, rdr) in enumerate(data):
                    prd = pprd.tile([128, CHUNK], f32, tag="prd")
                    nc.tensor.matmul(prd, lhsT=sel_sb, rhs=rdr,
                                     start=True, stop=True)
                    nc.vector.tensor_mul(aot[hp, j][0:64, :], oA[0:64, :],
                                         prd[0:64, :])
                    nc.vector.tensor_mul(aot[hp, j][64:128, :], shb[64:128, :],
                                         prd[64:128, :])
                for tt in range(4):
                    pout = ppo.tile([128, H], f32, tag="po", name=f"pout{j}_{tt}")
                    for ct2 in range(2):
                        nc.tensor.matmul(pout,
                                         lhsT=aot[ct2, j][:, 128 * tt:128 * (tt + 1)],
                                         rhs=woall[:, H * ct2:H * (ct2 + 1)],
                                         start=(ct2 == 0), stop=(ct2 == 1))
                    osb = ost.tile([128, H], bf, tag="ost")
                    nc.vector.tensor_copy(out=osb, in_=pout)
                    oeng = nc.sync if tt % 2 == 0 else nc.scalar
                    oeng.dma_start(
                        out=out_d[CHUNK * j + 128 * tt:CHUNK * j + 128 * (tt + 1), :],
                        in_=osb)

            prev = atth(0)
            for j in range(1, NCH):
                cur = atth(j)
                finish(j - 1, prev)
                prev = cur
            finish(NCH - 1, prev)

    nc.compile()
    _prog = nc
    return nc


def _host_inputs(x, Wq, bq, Wk, bk, Wv, bv, Wo, bo):
    """Build the 8 per-core input maps (packed mega-tensors, hardcoded shapes)."""
    x = np.asarray(x, np.float32)
    Wq, bq = np.asarray(Wq, np.float32), np.asarray(bq, np.float32)
    Wk, bk = np.asarray(Wk, np.float32), np.asarray(bk, np.float32)
    Wv, bv = np.asarray(Wv, np.float32), np.asarray(bv, np.float32)
    Wo = np.asarray(Wo, np.float32)

    def rowpack(a, cols):
        """[R*128, cols] -> [128, R*cols] row-tiles side by side."""
        r = a.shape[0] // 128
        return np.concatenate([a[128 * i:128 * (i + 1)] for i in range(r)], axis=1)

    xt8_all, xtc8_all, xtcv_all = [], [], []
    for b in range(B):
        xt = np.ascontiguousarray(x[b].T)            # (512, 2048)
        xtc = np.zeros((H, NSC), np.float32)
        xtc[:, :NSC_RAW] = xt[:, POS]
        xt8_all.append(rowpack(xt, T).astype(F8))
        xtc8_all.append(rowpack(xtc, NSC).astype(F8))
        xtcv_all.append(rowpack(xtc, NSC).astype(BF))
    vones = np.zeros((128, NSTC), np.float32)
    for s in range(NSTC):
        nvalid = max(0, min(128, NSC_RAW - 128 * s))
        vones[:nvalid, s] = 1.0

    # rope tables (match reference fp32 math)
    inv = (1.0 / (THETA ** (np.arange(0, H, 2, dtype=np.float32) / H))).astype(np.float32)
    tpos = np.arange(T, dtype=np.float32)
    ang = tpos[:, None] * inv[None, :]
    cosf = np.cos(ang).astype(np.float32).T     # (256, T)
    sinf = np.sin(ang).astype(np.float32).T

    def drpack(W):
        """[512, 128] -> [128, 256] DoubleRow stationary blocks (2 passes)."""
        out = np.zeros((2, 128, 2, 128), np.float32)
        for p in range(2):
            for i in range(2):
                out[p, :, i, :] = W[256 * p + 128 * i:256 * p + 128 * i + 128, :]
        return out.reshape(2, 128, 256)  # [pass][128, 256]

    per_g = []
    for g in range(2):
        cols = slice(CPG * g, CPG * (g + 1))
        wq_g, wk_g = Wq[:, cols], Wk[:, cols]
        wv_a = np.zeros((H, 260), np.float32)
        for hh in range(4):
            wv_a[:, 65 * hh:65 * hh + 64] = Wv[:, CPG * g + 64 * hh:CPG * g + 64 * (hh + 1)]
        # wqk8: [q|k] x [ct] x [pass] 256-col blocks
        blocks = []
        for W in (wq_g, wk_g):
            for ct in range(2):
                dp = drpack(W[:, 128 * ct:128 * (ct + 1)])
                blocks.extend([dp[0], dp[1]])
        wqk8 = np.concatenate(blocks, axis=1)       # [128, 2048]

        pr = slice(128 * g, 128 * (g + 1))
        cos_g = np.repeat(cosf[pr], 2, axis=0)      # C: repeat pairs
        sin_g = np.repeat(sinf[pr], 2, axis=0).copy()
        sin_g[1::2] *= -1.0                         # S~: minus on ODD rows
        cosk_g = np.zeros((CPG, NSC), np.float32)
        sink_g = np.zeros((CPG, NSC), np.float32)
        cosk_g[:, :NSC_RAW] = cos_g[:, POS]
        sink_g[:, :NSC_RAW] = sin_g[:, POS]
        # per-ct interleave: [Cq ct | Sq ct]... then [Ck ct | Sk ct]...
        cq, sq = rowpack(cos_g, T), rowpack(sin_g, T)        # [128, 2T] each
        ck, sk = rowpack(cosk_g, NSC), rowpack(sink_g, NSC)  # [128, 2NSC]
        trig = np.concatenate([cq[:, :T], sq[:, :T], cq[:, T:], sq[:, T:],
                               ck[:, :NSC], sk[:, :NSC],
                               ck[:, NSC:], sk[:, NSC:]], axis=1)
        biases = np.stack([
            bq[cols][:128], bq[cols][128:],
            bk[cols][:128], bk[cols][128:],
        ], axis=1).astype(np.float32)
        per_g.append(dict(
            wqk8=wqk8.astype(F8),
            wv=rowpack(wv_a, 260).astype(BF),
            wo=rowpack(Wo[cols, :], H).astype(BF),
            trig=trig.astype(F8), biases=biases,
        ))

    # causal band masks in compacted coords: valid iff POS[s] <= t
    spos = np.full(NSC, _INF, np.int64)
    spos[:NSC_RAW] = POS
    bmask = np.zeros((128, len(_CROSSINGS) * CHUNK), np.float32)
    for n, (i, j) in enumerate(_CROSSINGS):
        rows = spos[128 * i:128 * (i + 1)]
        tcols = np.arange(CHUNK * j, CHUNK * (j + 1))
        bmask[:, CHUNK * n:CHUNK * (n + 1)] = (rows[:, None] <= tcols[None, :])

    sel = np.zeros((2, 128), BF)
    sel[0, :64] = 1.0
    sel[1, 64:] = 1.0

    perm = np.zeros((128, 128), np.float32)
    perm[np.arange(128), np.arange(128) ^ 1] = 1.0

    shared = dict(bmask=bmask.astype(F8), sel=sel, perm=perm.astype(BF),
                  vones=vones)
    in_maps = []
    for c in range(NCORES):
        b, g = c // 2, c % 2
        m = dict(xt8=xt8_all[b], xtc8=xtc8_all[b], xtcv=xtcv_all[b], **shared)
        m.update(per_g[g])
        in_maps.append(m)
    return in_maps


def run(inputs, trace=False):
    """Build+run; returns BassKernelResults (per-core partials in .results)."""
    from concourse.bass_utils import run_bass_kernel_spmd
    nc = _build_program()
    in_maps = _host_inputs(**inputs)
    res = run_bass_kernel_spmd(nc, in_maps, list(range(NCORES)), trace=trace)
    return res


def assemble(results, Wv, bv, Wo, bo):
    """Sum per-core partials + host-folded bias (bv went through Wo)."""
    bo2 = (np.asarray(bo, np.float32)
           + np.asarray(bv, np.float32) @ np.asarray(Wo, np.float32))
    out = np.empty((B, T, H), np.float32)
    for b in range(B):
        out[b] = (results[2 * b]["out"].astype(np.float32)
                  + results[2 * b + 1]["out"].astype(np.float32)
                  + bo2[None, :])
    return out


def kernel(x, Wq, bq, Wk, bk, Wv, bv, Wo, bo):
    res = run(dict(x=x, Wq=Wq, bq=bq, Wk=Wk, bk=bk, Wv=Wv, bv=bv, Wo=Wo, bo=bo))
    return assemble(res.results, Wv, bv, Wo, bo)


# revision 68
# speedup vs baseline: 1.0756x; 1.0756x over previous
"""Trainium2 Bass kernel for masked causal attention with RoPE (mgdt column masking).

Reference computation (B=4, T=2048, H=512, heads=8, D=64):
  q/k/v = x @ W + b;  RoPE(q, k) over full hidden dim (pairs of adjacent channels);
  scores = q k^T / sqrt(D) with causal tril mask plus fully-masked key columns
  at {4, 7, 10, ...} (period 3); softmax; out = (att @ v) @ Wo + bo.

Sharding: 8 cores = data-parallel over batch (4) x tensor-parallel over head
groups (2 x 4 heads). Each core computes a [T, H] partial of its batch's
output projection (Wo row-sharded); host sums the pair of partials + bo.

Key structural choices (v2):
  - KEY COMPACTION: the 682 fully-masked key columns are removed on the host
    (x^T gathered at the 1366 unmasked positions, padded to 1408).
  - q/k are computed TRANSPOSED as qT[c, t] (c on partitions) in FP8 (e4m3)
    with DoubleRow matmuls (2 K-subtiles of 128 per instruction -> half the
    PE passes of bf16). Softmax washes out the fp8 noise on the scores path;
    V stays bf16 (its quantization error would land directly on the output).
  - RoPE without the swapped second projection: qrot = (q+b)*C + P@u,
    u = (q+b)*S~ where S~ is the sin table pre-swapped/sign-folded on the
    host and P is a 128x128 pair-swap permutation matmul (engines cannot
    partition-step; one K=128 F=512 matmul replaces two DR passes).
    bv is folded through Wo on the host (bo2 = bo + bv @ Wo), so the V
    projection has no bias pass; the denominator ones-column is restored
    with a tiny broadcast copy of per-tile valid flags.
  - scores are computed transposed (sT[s, t] per head, K=64 contraction, two
    heads packed in one [128, .] PSUM via PE row-tiling) so softmax's
    s-reduction and att@v's s-contraction keep s on partitions.
  - p = exp(sT/8) with no max subtraction (|scores/8| < ~2 by construction).
  - V is augmented per head with a ones column -> att@v emits the softmax
    denominator as row 64 of its PSUM output for free.
  - Normalization: denominators gathered (DMA), reciprocal'd in one [128, 8]
    DVE op, scattered back, broadcast over partitions with a K=2 selector
    matmul, applied with ONE [128, 512] multiply per (chunk, head-pair).
  - Inputs are loaded in consumption order, chunked, split across both HW
    DMA queues (sync + scalar) so the PE starts within ~1us. Trig tables and
    band masks ship as fp8; the output partial ships as bf16.
"""

import sys

if "/opt/trn_rl_repo" not in sys.path:
    sys.path.insert(0, "/opt/trn_rl_repo")

import numpy as np
import ml_dtypes

B, T, H, NH, D = 4, 2048, 512, 8, 64
THETA = 10000.0
PERIOD, RET_ORDER = 3, 2
NCORES = 8
CPG = H // 2          # 256 channels per head-group shard
CHUNK = 512           # t-chunk (one PSUM bank of fp32)
NCH = T // CHUNK      # 4 query chunks
BF = ml_dtypes.bfloat16
F8 = ml_dtypes.float8_e4m3

# --- compacted key geometry (host + builder share this) ---
_cm = np.ones(T, bool)
_cm[PERIOD + RET_ORDER - 1::PERIOD] = False
POS = np.where(_cm)[0]              # 1366 unmasked key positions
NSC_RAW = len(POS)                  # 1366
NSTC = (NSC_RAW + 127) // 128       # 11 s-tiles
NSC = NSTC * 128                    # 1408 padded
KCW = [CHUNK, CHUNK, NSC - 2 * CHUNK]   # k-projection chunk widths (512,512,384)

# per s-tile first/last valid original positions
_INF = 1 << 30
TILE_LO = [int(POS[128 * i]) if 128 * i < NSC_RAW else _INF for i in range(NSTC)]
TILE_HI = [int(POS[min(128 * i + 127, NSC_RAW - 1)]) if 128 * i < NSC_RAW else _INF
           for i in range(NSTC)]


def _tiles_for_chunk(j):
    """(i, col0, crossing) for each compact s-tile contributing to t-chunk j."""
    out = []
    for i in range(NSTC):
        lo, hi = TILE_LO[i], TILE_HI[i]
        if lo > CHUNK * j + CHUNK - 1:
            continue
        col0 = max(0, lo - CHUNK * j)
        crossing = hi > CHUNK * j  # some (row, col) pairs invalid -> needs mask
        out.append((i, col0, crossing))
    return out


_CROSSINGS = sorted({(i, j) for j in range(NCH)
                     for (i, c0, cr) in _tiles_for_chunk(j) if cr})

_prog = None


def _build_program():
    global _prog
    if _prog is not None:
        return _prog
    from contextlib import ExitStack
    import concourse.bacc as bacc
    import concourse.tile as tile
    import concourse.bass as _bass
    from concourse import mybir

    bf = mybir.dt.bfloat16
    f8 = mybir.dt.float8e4
    f32 = mybir.dt.float32
    EXP = mybir.ActivationFunctionType.Exp
    ADD = mybir.AluOpType.add
    MULT = mybir.AluOpType.mult
    DR = mybir.MatmulPerfMode.DoubleRow

    nc = bacc.Bacc("TRN2", target_bir_lowering=False, debug=False, num_devices=NCORES)

    def din(name, shape, dt):
        return nc.dram_tensor(name, shape, dt, kind="ExternalInput").ap()

    nm = len(_CROSSINGS)
    xt8_d = din("xt8", [128, 4 * T], f8)           # x^T 4 row-tiles side by side
    xtc8_d = din("xtc8", [128, 4 * NSC], f8)       # compacted x^T row-tiles
    xtcv_d = din("xtcv", [128, 4 * NSC], bf)       # bf16 copy for V projection
    wqk8_d = din("wqk8", [128, 2048], f8)          # q|k x ct x pass, 256-col blocks
    wv_d = din("wv", [128, 4 * 260], bf)
    wo_d = din("wo", [128, 2 * H], bf)
    # trig layout: [Cq ct0 | Sq~ ct0 | Cq ct1 | Sq~ ct1 | Ck ct0 | Sk~ ct0 |
    #               Ck ct1 | Sk~ ct1] so each ct's cos+sin are adjacent
    trig_d = din("trig", [128, 2 * (2 * T + 2 * NSC)], f8)
    bm_d = din("bmask", [128, nm * CHUNK], f8)
    sel_d = din("sel", [2, 128], bf)
    perm_d = din("perm", [128, 128], bf)           # partition pair-swap matrix
    vone_d = din("vones", [128, NSTC], f32)        # per s-tile valid-row flags
    bias_d = din("biases", [128, 4], f32)          # bq ct0|bq ct1|bk ct0|bk ct1
    out_d = nc.dram_tensor("out", [T, H], bf, kind="ExternalOutput").ap()

    TRIG_K = 4 * T

    with tile.TileContext(nc) as tc:
        with ExitStack() as ctx:
            sg = ctx.enter_context(tc.tile_pool(name="sg", bufs=1))

            def ld(name, dram, cols, dt=bf, nsplit=4, eng=None):
                eng = eng or nc.sync
                tl = sg.tile([128, cols], dt, tag=name, name=name)
                step = -(-cols // nsplit)
                for a in range(0, cols, step):
                    b = min(a + step, cols)
                    eng.dma_start(out=tl[:, a:b], in_=dram[:, a:b])
                return tl

            # ---- input loads in consumption order, split across 2 HW queues
            # sync queue: only the big projection inputs, first-use first;
            # fewer chunks = fewer 0.6us issue slots + less DMA-sem recycling
            wqk8 = sg.tile([128, 2048], f8, tag="wqk8", name="wqk8")
            nc.sync.dma_start(out=wqk8[:, 1024:2048], in_=wqk8_d[:, 1024:2048])
            xtc8 = ld("xtc8", xtc8_d, 4 * NSC, dt=f8, nsplit=2)
            trigk = sg.tile([128, 4 * NSC], f8, tag="trigk", name="trigk")
            for a in range(0, 4 * NSC, 2 * NSC):
                nc.sync.dma_start(out=trigk[:, a:a + 2 * NSC],
                                  in_=trig_d[:, TRIG_K + a:TRIG_K + a + 2 * NSC])
            nc.sync.dma_start(out=wqk8[:, 0:1024], in_=wqk8_d[:, 0:1024])
            xt8 = ld("xt8", xt8_d, 4 * T, dt=f8, nsplit=2)
            trigq = sg.tile([128, 4 * T], f8, tag="trigq", name="trigq")
            for a in range(0, 4 * T, 2 * T):
                nc.sync.dma_start(out=trigq[:, a:a + 2 * T],
                                  in_=trig_d[:, a:a + 2 * T])
            # scalar queue: small constants, then v projection + attention inputs
            bias_sb = sg.tile([128, 4], f32, tag="biases")
            nc.scalar.dma_start(out=bias_sb, in_=bias_d[:, :])
            perm_sb = sg.tile([128, 128], bf, tag="perm")
            nc.scalar.dma_start(out=perm_sb, in_=perm_d[:, :])
            sel_sb = sg.tile([2, 128], bf, tag="sel")
            nc.scalar.dma_start(out=sel_sb, in_=sel_d[:, :])
            vone_sb = sg.tile([128, NSTC], f32, tag="vones")
            nc.scalar.dma_start(out=vone_sb, in_=vone_d[:, :])
            xtcv = ld("xtcv", xtcv_d, 4 * NSC, dt=bf, nsplit=2, eng=nc.scalar)
            wvall = ld("wvall", wv_d, 4 * 260, dt=bf, nsplit=1, eng=nc.scalar)
            bmall = ld("bmall", bm_d, nm * CHUNK, dt=f8, nsplit=1, eng=nc.scalar)
            woall = ld("woall", wo_d, 2 * H, dt=bf, nsplit=1, eng=nc.scalar)

            def wqk(v, ct, p):
                """stationary [128, 2, 128] for variant v (0=q, 1=k), ct, pass p."""
                base = 1024 * v + 512 * ct + 256 * p
                t = wqk8[:, base:base + 256]
                return _bass.AP(tensor=t.tensor, offset=t.offset,
                                ap=[t.ap[0], [128, 2], [1, 128]])

            def dr_rhs(xtile, tile_cols, p, csl):
                """moving [128, 2, w] = row-tile pair (2p, 2p+1), cols csl."""
                t = xtile[:, tile_cols * 2 * p + csl.start:
                          tile_cols * 2 * p + csl.stop]
                return _bass.AP(tensor=t.tensor, offset=t.offset,
                                ap=[t.ap[0], [tile_cols, 2],
                                    [1, csl.stop - csl.start]])

            def bm_slice(i, j, c0):
                n = _CROSSINGS.index((i, j))
                return bmall[:, CHUNK * n + c0:CHUNK * (n + 1)]

            # persistent activations
            qrot, krot, aot = {}, {}, {}
            for ct in range(2):
                for ch in range(NCH):
                    qrot[ct, ch] = sg.tile([128, CHUNK], bf, tag=f"qr{ct}_{ch}",
                                           name=f"qr{ct}_{ch}")
                    aot[ct, ch] = sg.tile([128, CHUNK], bf, tag=f"ao{ct}_{ch}",
                                          name=f"ao{ct}_{ch}")
                for kc in range(3):
                    krot[ct, kc] = sg.tile([128, CHUNK], bf, tag=f"kr{ct}_{kc}",
                                           name=f"kr{ct}_{kc}")
            vaug = []
            for s in range(NSTC):
                vaug.append(sg.tile([128, 260], bf, tag=f"va{s}", name=f"va{s}"))

            pp = ctx.enter_context(tc.tile_pool(name="pp", bufs=8))
            rtmp = ctx.enter_context(tc.tile_pool(name="rtmp", bufs=4))
            dn = ctx.enter_context(tc.tile_pool(name="dn", bufs=4))
            stg = ctx.enter_context(tc.tile_pool(name="stg", bufs=4))
            ost = ctx.enter_context(tc.tile_pool(name="ost", bufs=4))

            # ---- phase B: K/V/Q projections + rope ----
            with tc.tile_pool(name="ppj", bufs=4, space="PSUM") as ppj, \
                 tc.tile_pool(name="ppw", bufs=2, space="PSUM") as ppw, \
                 tc.tile_pool(name="ppv", bufs=2, space="PSUM") as ppv:

                def proj_stage1(ct, var, bcol, dst, xtile, tile_cols, csl,
                                csb, ssb, tag):
                    """pm = x8 @ W8 (fp8 DoubleRow); dst = (pm+b)*C; u = (pm+b)*S~."""
                    w = csl.stop - csl.start
                    ps = ppj.tile([128, CHUNK], f32, tag="ps", name=f"pj_{tag}")
                    pm = ps[:, 0:w]
                    for p in range(2):
                        nc.tensor.matmul(pm, lhsT=wqk(var, ct, p),
                                         rhs=dr_rhs(xtile, tile_cols, p, csl),
                                         start=(p == 0), stop=(p == 1),
                                         perf_mode=DR)
                    nc.vector.scalar_tensor_tensor(
                        out=dst[:, :w], in0=pm, scalar=bias_sb[:, bcol:bcol + 1],
                        in1=csb, op0=ADD, op1=MULT)
                    u = rtmp.tile([128, CHUNK], bf, tag="u")
                    nc.vector.scalar_tensor_tensor(
                        out=u[:, :w], in0=pm, scalar=bias_sb[:, bcol:bcol + 1],
                        in1=ssb, op0=ADD, op1=MULT)
                    return dst, u, w, tag

                def proj_stage2(dst, u, w, tag):
                    """dst += P @ u (partition-pair swap via perm matmul)."""
                    psw = ppw.tile([128, CHUNK], f32, tag="psw",
                                   name=f"psw_{tag}")
                    nc.tensor.matmul(psw[:, 0:w], lhsT=perm_sb, rhs=u[:, :w],
                                     start=True, stop=True)
                    nc.vector.tensor_add(dst[:, :w], dst[:, :w], psw[:, 0:w])

                pend = []
                for ct in range(2):
                    for kc, w in enumerate(KCW):
                        csl = slice(CHUNK * kc, CHUNK * kc + w)
                        if len(pend) >= 2:
                            proj_stage2(*pend.pop(0))
                        pend.append(proj_stage1(
                            ct, 1, 2 + ct, krot[ct, kc], xtc8, NSC, csl,
                            trigk[:, 2 * NSC * ct + csl.start:
                                  2 * NSC * ct + csl.stop],
                            trigk[:, 2 * NSC * ct + NSC + csl.start:
                                  2 * NSC * ct + NSC + csl.stop],
                            f"k{ct}_{kc}"))
                for s in range(NSTC):
                    ssl = slice(128 * s, 128 * (s + 1))
                    pv = ppv.tile([128, 260], f32, tag="pv", name=f"pv{s}")
                    for kt in range(4):
                        nc.tensor.matmul(pv, lhsT=xtcv[:, NSC * kt + ssl.start:
                                                       NSC * kt + ssl.stop],
                                         rhs=wvall[:, 260 * kt:260 * (kt + 1)],
                                         start=(kt == 0), stop=(kt == 3))
                    nc.scalar.copy(out=vaug[s], in_=pv)
                    # denominator ones-column (0 at pad rows), bv is folded
                    # into the host-side bias via bv @ Wo
                    vo = vone_sb[:, s:s + 1]
                    vob = _bass.AP(tensor=vo.tensor, offset=vo.offset,
                                   ap=[vo.ap[0], [0, 4]])
                    nc.vector.tensor_copy(out=vaug[s][:, 64:260:65], in_=vob)
                    if pend and s % 2 == 1:
                        proj_stage2(*pend.pop(0))
                for j in range(NCH):
                    for ct in range(2):
                        csl = slice(CHUNK * j, CHUNK * (j + 1))
                        if len(pend) >= 2:
                            proj_stage2(*pend.pop(0))
                        pend.append(proj_stage1(
                            ct, 0, ct, qrot[ct, j], xt8, T, csl,
                            trigq[:, 2 * T * ct + csl.start:
                                  2 * T * ct + csl.stop],
                            trigq[:, 2 * T * ct + T + csl.start:
                                  2 * T * ct + T + csl.stop],
                            f"q{ct}_{j}"))
                while pend:
                    proj_stage2(*pend.pop(0))

            pps = ctx.enter_context(
                tc.tile_pool(name="pps", bufs=2, space="PSUM"))
            ppo = ctx.enter_context(tc.tile_pool(name="ppo", bufs=3, space="PSUM"))
            pprd = ctx.enter_context(tc.tile_pool(name="pprd", bufs=1, space="PSUM"))

            # ---- attention + output projection ----
            def atth(j):
                """scores/exp/mask/attv for both head pairs of chunk j, plus
                PSUM->SBUF staging and the reciprocal-denominator path.
                Returns per-hp staging handles for finish(j)."""
                tiles_j = _tiles_for_chunk(j)
                ret = []
                for hp in range(2):
                    ct = hp
                    po = [ppo.tile([65, CHUNK], f32, tag="po",
                                   name=f"po{j}_{hp}_{i}") for i in range(2)]
                    for si, (s, col0, crossing) in enumerate(tiles_j):
                        first = si == 0
                        last = si == len(tiles_j) - 1
                        ksl = slice(128 * (s % 4), 128 * (s % 4) + 128)
                        ps = pps.tile([128, 2 * CHUNK], f32, tag="ps",
                                      name=f"ps{j}_{hp}_{s}")
                        for idx in range(2):
                            pb = 64 * idx
                            nc.tensor.matmul(
                                ps[:, CHUNK * idx + col0:CHUNK * (idx + 1)],
                                lhsT=krot[ct, s // 4][pb:pb + 64, ksl],
                                rhs=qrot[ct, j][pb:pb + 64, col0:],
                                start=True, stop=True)
                        pt = pp.tile([128, 2 * CHUNK], bf, tag="p",
                                     name=f"pt{j}_{hp}_{s}")
                        if col0:
                            for idx in range(2):
                                csl2 = slice(CHUNK * idx + col0, CHUNK * (idx + 1))
                                nc.scalar.activation(out=pt[:, csl2],
                                                     in_=ps[:, csl2],
                                                     func=EXP, scale=0.125)
                        else:
                            nc.scalar.activation(out=pt, in_=ps,
                                                 func=EXP, scale=0.125)
                        if crossing:
                            w2 = CHUNK - col0
                            base = pt[:, col0:col0 + w2]
                            pt2 = _bass.AP(tensor=base.tensor, offset=base.offset,
                                           ap=[base.ap[0], [CHUNK, 2], [1, w2]])
                            bmb = bm_slice(s, j, col0)
                            bm2 = _bass.AP(tensor=bmb.tensor, offset=bmb.offset,
                                           ap=[bmb.ap[0], [0, 2], [1, w2]])
                            nc.vector.tensor_mul(pt2, pt2, bm2)
                        for idx in range(2):
                            hh = 2 * hp + idx
                            nc.tensor.matmul(
                                po[idx][:, col0:],
                                lhsT=vaug[s][:, 65 * hh:65 * hh + 65],
                                rhs=pt[:, CHUNK * idx + col0:CHUNK * (idx + 1)],
                                start=first, stop=last,
                                skip_group_check=True)
                    # move PSUM results to SBUF (DMA cannot read PSUM)
                    oA = stg.tile([65, CHUNK], f32, tag="oA")
                    oB = stg.tile([65, CHUNK], f32, tag="oB")
                    nc.vector.tensor_copy(out=oA, in_=po[0])
                    nc.vector.tensor_copy(out=oB, in_=po[1])
                    shb = stg.tile([128, CHUNK], f32, tag="shb")
                    nc.sync.dma_start(out=shb[64:128, :], in_=oB[0:64, :])
                    dsb = dn.tile([128, 8], f32, tag="den")
                    nc.sync.dma_start(out=dsb[:, 0:4], in_=oA[64:65, :])
                    nc.sync.dma_start(out=dsb[:, 4:8], in_=oB[64:65, :])
                    rsb = dn.tile([128, 8], bf, tag="rden")
                    with nc.allow_low_precision(reason="bf16 softmax recip"):
                        nc.vector.reciprocal(rsb, dsb)
                    rdr = dn.tile([2, CHUNK], bf, tag="rdr")
                    nc.sync.dma_start(out=rdr[0:1, :], in_=rsb[:, 0:4])
                    nc.sync.dma_start(out=rdr[1:2, :], in_=rsb[:, 4:8])
                    ret.append((oA, shb, rdr))
                return ret

            def finish(j, data):
                """normalize chunk j (deferred past atth(j+1)) + out-projection."""
                for hp, (oA, shb, rdr) in enumerate(data):
                    prd = pprd.tile([128, CHUNK], f32, tag="prd")
                    nc.tensor.matmul(prd, lhsT=sel_sb, rhs=rdr,
                                     start=True, stop=True)
                    nc.vector.tensor_mul(aot[hp, j][0:64, :], oA[0:64, :],
                                         prd[0:64, :])
                    nc.vector.tensor_mul(aot[hp, j][64:128, :], shb[64:128, :],
                                         prd[64:128, :])
                for tt in range(4):
                    pout = ppo.tile([128, H], f32, tag="po", name=f"pout{j}_{tt}")
                    for ct2 in range(2):
                        nc.tensor.matmul(pout,
                                         lhsT=aot[ct2, j][:, 128 * tt:128 * (tt + 1)],
                                         rhs=woall[:, H * ct2:H * (ct2 + 1)],
                                         start=(ct2 == 0), stop=(ct2 == 1))
                    osb = ost.tile([128, H], bf, tag="ost")
                    nc.vector.tensor_copy(out=osb, in_=pout)
                    oeng = nc.sync if tt % 2 == 0 else nc.scalar
                    oeng.dma_start(
                        out=out_d[CHUNK * j + 128 * tt:CHUNK * j + 128 * (tt + 1), :],
                        in_=osb)

            prev = atth(0)
            for j in range(1, NCH):
                cur = atth(j)
                finish(j - 1, prev)
                prev = cur
            finish(NCH - 1, prev)

    nc.compile()
    _prog = nc
    return nc


def _host_inputs(x, Wq, bq, Wk, bk, Wv, bv, Wo, bo):
    """Build the 8 per-core input maps (packed mega-tensors, hardcoded shapes)."""
    x = np.asarray(x, np.float32)
    Wq, bq = np.asarray(Wq, np.float32), np.asarray(bq, np.float32)
    Wk, bk = np.asarray(Wk, np.float32), np.asarray(bk, np.float32)
    Wv, bv = np.asarray(Wv, np.float32), np.asarray(bv, np.float32)
    Wo = np.asarray(Wo, np.float32)

    def rowpack(a, cols):
        """[R*128, cols] -> [128, R*cols] row-tiles side by side."""
        r = a.shape[0] // 128
        return np.concatenate([a[128 * i:128 * (i + 1)] for i in range(r)], axis=1)

    xt8_all, xtc8_all, xtcv_all = [], [], []
    for b in range(B):
        xt = np.ascontiguousarray(x[b].T)            # (512, 2048)
        xtc = np.zeros((H, NSC), np.float32)
        xtc[:, :NSC_RAW] = xt[:, POS]
        xt8_all.append(rowpack(xt, T).astype(F8))
        xtc8_all.append(rowpack(xtc, NSC).astype(F8))
        xtcv_all.append(rowpack(xtc, NSC).astype(BF))
    vones = np.zeros((128, NSTC), np.float32)
    for s in range(NSTC):
        nvalid = max(0, min(128, NSC_RAW - 128 * s))
        vones[:nvalid, s] = 1.0

    # rope tables (match reference fp32 math)
    inv = (1.0 / (THETA ** (np.arange(0, H, 2, dtype=np.float32) / H))).astype(np.float32)
    tpos = np.arange(T, dtype=np.float32)
    ang = tpos[:, None] * inv[None, :]
    cosf = np.cos(ang).astype(np.float32).T     # (256, T)
    sinf = np.sin(ang).astype(np.float32).T

    def drpack(W):
        """[512, 128] -> [128, 256] DoubleRow stationary blocks (2 passes)."""
        out = np.zeros((2, 128, 2, 128), np.float32)
        for p in range(2):
            for i in range(2):
                out[p, :, i, :] = W[256 * p + 128 * i:256 * p + 128 * i + 128, :]
        return out.reshape(2, 128, 256)  # [pass][128, 256]

    per_g = []
    for g in range(2):
        cols = slice(CPG * g, CPG * (g + 1))
        wq_g, wk_g = Wq[:, cols], Wk[:, cols]
        wv_a = np.zeros((H, 260), np.float32)
        for hh in range(4):
            wv_a[:, 65 * hh:65 * hh + 64] = Wv[:, CPG * g + 64 * hh:CPG * g + 64 * (hh + 1)]
        # wqk8: [q|k] x [ct] x [pass] 256-col blocks
        blocks = []
        for W in (wq_g, wk_g):
            for ct in range(2):
                dp = drpack(W[:, 128 * ct:128 * (ct + 1)])
                blocks.extend([dp[0], dp[1]])
        wqk8 = np.concatenate(blocks, axis=1)       # [128, 2048]

        pr = slice(128 * g, 128 * (g + 1))
        cos_g = np.repeat(cosf[pr], 2, axis=0)      # C: repeat pairs
        sin_g = np.repeat(sinf[pr], 2, axis=0).copy()
        sin_g[1::2] *= -1.0                         # S~: minus on ODD rows
        cosk_g = np.zeros((CPG, NSC), np.float32)
        sink_g = np.zeros((CPG, NSC), np.float32)
        cosk_g[:, :NSC_RAW] = cos_g[:, POS]
        sink_g[:, :NSC_RAW] = sin_g[:, POS]
        # per-ct interleave: [Cq ct | Sq ct]... then [Ck ct | Sk ct]...
        cq, sq = rowpack(cos_g, T), rowpack(sin_g, T)        # [128, 2T] each
        ck, sk = rowpack(cosk_g, NSC), rowpack(sink_g, NSC)  # [128, 2NSC]
        trig = np.concatenate([cq[:, :T], sq[:, :T], cq[:, T:], sq[:, T:],
                               ck[:, :NSC], sk[:, :NSC],
                               ck[:, NSC:], sk[:, NSC:]], axis=1)
        biases = np.stack([
            bq[cols][:128], bq[cols][128:],
            bk[cols][:128], bk[cols][128:],
        ], axis=1).astype(np.float32)
        per_g.append(dict(
            wqk8=wqk8.astype(F8),
            wv=rowpack(wv_a, 260).astype(BF),
            wo=rowpack(Wo[cols, :], H).astype(BF),
            trig=trig.astype(F8), biases=biases,
        ))

    # causal band masks in compacted coords: valid iff POS[s] <= t
    spos = np.full(NSC, _INF, np.int64)
    spos[:NSC_RAW] = POS
    bmask = np.zeros((128, len(_CROSSINGS) * CHUNK), np.float32)
    for n, (i, j) in enumerate(_CROSSINGS):
        rows = spos[128 * i:128 * (i + 1)]
        tcols = np.arange(CHUNK * j, CHUNK * (j + 1))
        bmask[:, CHUNK * n:CHUNK * (n + 1)] = (rows[:, None] <= tcols[None, :])

    sel = np.zeros((2, 128), BF)
    sel[0, :64] = 1.0
    sel[1, 64:] = 1.0

    perm = np.zeros((128, 128), np.float32)
    perm[np.arange(128), np.arange(128) ^ 1] = 1.0

    shared = dict(bmask=bmask.astype(F8), sel=sel, perm=perm.astype(BF),
                  vones=vones)
    in_maps = []
    for c in range(NCORES):
        b, g = c // 2, c % 2
        m = dict(xt8=xt8_all[b], xtc8=xtc8_all[b], xtcv=xtcv_all[b], **shared)
        m.update(per_g[g])
        in_maps.append(m)
    return in_maps


def run(inputs, trace=False):
    """Build+run; returns BassKernelResults (per-core partials in .results)."""
    from concourse.bass_utils import run_bass_kernel_spmd
    nc = _build_program()
    in_maps = _host_inputs(**inputs)
    res = run_bass_kernel_spmd(nc, in_maps, list(range(NCORES)), trace=trace)
    return res


def assemble(results, Wv, bv, Wo, bo):
    """Sum per-core partials + host-folded bias (bv went through Wo)."""
    bo2 = (np.asarray(bo, np.float32)
           + np.asarray(bv, np.float32) @ np.asarray(Wo, np.float32))
    out = np.empty((B, T, H), np.float32)
    for b in range(B):
        out[b] = (results[2 * b]["out"].astype(np.float32)
                  + results[2 * b + 1]["out"].astype(np.float32)
                  + bo2[None, :])
    return out


def kernel(x, Wq, bq, Wk, bk, Wv, bv, Wo, bo):
    res = run(dict(x=x, Wq=Wq, bq=bq, Wk=Wk, bk=bk, Wv=Wv, bv=bv, Wo=Wo, bo=bo))
    return assemble(res.results, Wv, bv, Wo, bo)


# revision 69
# speedup vs baseline: 1.0799x; 1.0041x over previous
"""Trainium2 Bass kernel for masked causal attention with RoPE (mgdt column masking).

Reference computation (B=4, T=2048, H=512, heads=8, D=64):
  q/k/v = x @ W + b;  RoPE(q, k) over full hidden dim (pairs of adjacent channels);
  scores = q k^T / sqrt(D) with causal tril mask plus fully-masked key columns
  at {4, 7, 10, ...} (period 3); softmax; out = (att @ v) @ Wo + bo.

Sharding: 8 cores = data-parallel over batch (4) x tensor-parallel over head
groups (2 x 4 heads). Each core computes a [T, H] partial of its batch's
output projection (Wo row-sharded); host sums the pair of partials + bo.

Key structural choices (v2):
  - KEY COMPACTION: the 682 fully-masked key columns are removed on the host
    (x^T gathered at the 1366 unmasked positions, padded to 1408).
  - q/k are computed TRANSPOSED as qT[c, t] (c on partitions) in FP8 (e4m3)
    with DoubleRow matmuls (2 K-subtiles of 128 per instruction -> half the
    PE passes of bf16). Softmax washes out the fp8 noise on the scores path;
    V stays bf16 (its quantization error would land directly on the output).
  - RoPE without the swapped second projection: qrot = (q+b)*C + P@u,
    u = (q+b)*S~ where S~ is the sin table pre-swapped/sign-folded on the
    host and P is a 128x128 pair-swap permutation matmul (engines cannot
    partition-step; one K=128 F=512 matmul replaces two DR passes).
    bv is folded through Wo on the host (bo2 = bo + bv @ Wo), so the V
    projection has no bias pass; the denominator ones-column is restored
    with a tiny broadcast copy of per-tile valid flags.
  - scores are computed transposed (sT[s, t] per head, K=64 contraction, two
    heads packed in one [128, .] PSUM via PE row-tiling) so softmax's
    s-reduction and att@v's s-contraction keep s on partitions.
  - p = exp(sT/8) with no max subtraction (|scores/8| < ~2 by construction).
  - V is augmented per head with a ones column -> att@v emits the softmax
    denominator as row 64 of its PSUM output for free.
  - Normalization: denominators gathered (DMA), reciprocal'd in one [128, 8]
    DVE op, scattered back, broadcast over partitions with a K=2 selector
    matmul, applied with ONE [128, 512] multiply per (chunk, head-pair).
  - Inputs are loaded in consumption order, chunked, split across both HW
    DMA queues (sync + scalar) so the PE starts within ~1us. Trig tables and
    band masks ship as fp8; the output partial ships as bf16.
"""

import sys

if "/opt/trn_rl_repo" not in sys.path:
    sys.path.insert(0, "/opt/trn_rl_repo")

import numpy as np
import ml_dtypes

B, T, H, NH, D = 4, 2048, 512, 8, 64
THETA = 10000.0
PERIOD, RET_ORDER = 3, 2
NCORES = 8
CPG = H // 2          # 256 channels per head-group shard
CHUNK = 512           # t-chunk (one PSUM bank of fp32)
NCH = T // CHUNK      # 4 query chunks
BF = ml_dtypes.bfloat16
F8 = ml_dtypes.float8_e4m3

# --- compacted key geometry (host + builder share this) ---
_cm = np.ones(T, bool)
_cm[PERIOD + RET_ORDER - 1::PERIOD] = False
POS = np.where(_cm)[0]              # 1366 unmasked key positions
NSC_RAW = len(POS)                  # 1366
NSTC = (NSC_RAW + 127) // 128       # 11 s-tiles
NSC = NSTC * 128                    # 1408 padded
KCW = [CHUNK, CHUNK, NSC - 2 * CHUNK]   # k-projection chunk widths (512,512,384)

# per s-tile first/last valid original positions
_INF = 1 << 30
TILE_LO = [int(POS[128 * i]) if 128 * i < NSC_RAW else _INF for i in range(NSTC)]
TILE_HI = [int(POS[min(128 * i + 127, NSC_RAW - 1)]) if 128 * i < NSC_RAW else _INF
           for i in range(NSTC)]


def _tiles_for_chunk(j):
    """(i, col0, crossing) for each compact s-tile contributing to t-chunk j."""
    out = []
    for i in range(NSTC):
        lo, hi = TILE_LO[i], TILE_HI[i]
        if lo > CHUNK * j + CHUNK - 1:
            continue
        col0 = max(0, lo - CHUNK * j)
        crossing = hi > CHUNK * j  # some (row, col) pairs invalid -> needs mask
        out.append((i, col0, crossing))
    return out


_CROSSINGS = sorted({(i, j) for j in range(NCH)
                     for (i, c0, cr) in _tiles_for_chunk(j) if cr})

_prog = None


def _build_program():
    global _prog
    if _prog is not None:
        return _prog
    from contextlib import ExitStack
    import concourse.bacc as bacc
    import concourse.tile as tile
    import concourse.bass as _bass
    from concourse import mybir

    bf = mybir.dt.bfloat16
    f8 = mybir.dt.float8e4
    f32 = mybir.dt.float32
    EXP = mybir.ActivationFunctionType.Exp
    ADD = mybir.AluOpType.add
    MULT = mybir.AluOpType.mult
    DR = mybir.MatmulPerfMode.DoubleRow

    nc = bacc.Bacc("TRN2", target_bir_lowering=False, debug=False, num_devices=NCORES)

    def din(name, shape, dt):
        return nc.dram_tensor(name, shape, dt, kind="ExternalInput").ap()

    nm = len(_CROSSINGS)
    xt8_d = din("xt8", [128, 4 * T], f8)           # x^T 4 row-tiles side by side
    xtc8_d = din("xtc8", [128, 4 * NSC], f8)       # compacted x^T row-tiles
    xtcv_d = din("xtcv", [128, 4 * NSC], bf)       # bf16 copy for V projection
    wqk8_d = din("wqk8", [128, 2048], f8)          # q|k x ct x pass, 256-col blocks
    wv_d = din("wv", [128, 4 * 260], bf)
    wo_d = din("wo", [128, 2 * H], bf)
    # trig layout: [Cq ct0 | Sq~ ct0 | Cq ct1 | Sq~ ct1 | Ck ct0 | Sk~ ct0 |
    #               Ck ct1 | Sk~ ct1] so each ct's cos+sin are adjacent
    trig_d = din("trig", [128, 2 * (2 * T + 2 * NSC)], f8)
    bm_d = din("bmask", [128, nm * CHUNK], f8)
    sel_d = din("sel", [2, 128], bf)
    perm_d = din("perm", [128, 128], bf)           # partition pair-swap matrix
    vone_d = din("vones", [128, NSTC], f32)        # per s-tile valid-row flags
    bias_d = din("biases", [128, 4], f32)          # bq ct0|bq ct1|bk ct0|bk ct1
    out_d = nc.dram_tensor("out", [T, H], bf, kind="ExternalOutput").ap()

    TRIG_K = 4 * T

    with tile.TileContext(nc) as tc:
        with ExitStack() as ctx:
            sg = ctx.enter_context(tc.tile_pool(name="sg", bufs=1))

            def ld(name, dram, cols, dt=bf, nsplit=4, eng=None):
                eng = eng or nc.sync
                tl = sg.tile([128, cols], dt, tag=name, name=name)
                step = -(-cols // nsplit)
                for a in range(0, cols, step):
                    b = min(a + step, cols)
                    eng.dma_start(out=tl[:, a:b], in_=dram[:, a:b])
                return tl

            # ---- input loads in consumption order, split across 2 HW queues
            # sync queue: only the big projection inputs, first-use first;
            # fewer chunks = fewer 0.6us issue slots + less DMA-sem recycling
            wqk8 = sg.tile([128, 2048], f8, tag="wqk8", name="wqk8")
            nc.sync.dma_start(out=wqk8[:, 1024:2048], in_=wqk8_d[:, 1024:2048])
            xtc8 = ld("xtc8", xtc8_d, 4 * NSC, dt=f8, nsplit=2)
            trigk = sg.tile([128, 4 * NSC], f8, tag="trigk", name="trigk")
            for a in range(0, 4 * NSC, 2 * NSC):
                nc.sync.dma_start(out=trigk[:, a:a + 2 * NSC],
                                  in_=trig_d[:, TRIG_K + a:TRIG_K + a + 2 * NSC])
            nc.sync.dma_start(out=wqk8[:, 0:1024], in_=wqk8_d[:, 0:1024])
            xt8 = ld("xt8", xt8_d, 4 * T, dt=f8, nsplit=2)
            trigq = sg.tile([128, 4 * T], f8, tag="trigq", name="trigq")
            for a in range(0, 4 * T, 2 * T):
                nc.sync.dma_start(out=trigq[:, a:a + 2 * T],
                                  in_=trig_d[:, a:a + 2 * T])
            # scalar queue: small constants, then v projection + attention inputs
            bias_sb = sg.tile([128, 4], f32, tag="biases")
            nc.scalar.dma_start(out=bias_sb, in_=bias_d[:, :])
            perm_sb = sg.tile([128, 128], bf, tag="perm")
            nc.scalar.dma_start(out=perm_sb, in_=perm_d[:, :])
            sel_sb = sg.tile([2, 128], bf, tag="sel")
            nc.scalar.dma_start(out=sel_sb, in_=sel_d[:, :])
            vone_sb = sg.tile([128, NSTC], f32, tag="vones")
            nc.scalar.dma_start(out=vone_sb, in_=vone_d[:, :])
            xtcv = ld("xtcv", xtcv_d, 4 * NSC, dt=bf, nsplit=2, eng=nc.scalar)
            wvall = ld("wvall", wv_d, 4 * 260, dt=bf, nsplit=1, eng=nc.scalar)
            bmall = ld("bmall", bm_d, nm * CHUNK, dt=f8, nsplit=1, eng=nc.scalar)
            woall = ld("woall", wo_d, 2 * H, dt=bf, nsplit=1, eng=nc.scalar)

            def wqk(v, ct, p):
                """stationary [128, 2, 128] for variant v (0=q, 1=k), ct, pass p."""
                base = 1024 * v + 512 * ct + 256 * p
                t = wqk8[:, base:base + 256]
                return _bass.AP(tensor=t.tensor, offset=t.offset,
                                ap=[t.ap[0], [128, 2], [1, 128]])

            def dr_rhs(xtile, tile_cols, p, csl):
                """moving [128, 2, w] = row-tile pair (2p, 2p+1), cols csl."""
                t = xtile[:, tile_cols * 2 * p + csl.start:
                          tile_cols * 2 * p + csl.stop]
                return _bass.AP(tensor=t.tensor, offset=t.offset,
                                ap=[t.ap[0], [tile_cols, 2],
                                    [1, csl.stop - csl.start]])

            def bm_slice(i, j, c0):
                n = _CROSSINGS.index((i, j))
                return bmall[:, CHUNK * n + c0:CHUNK * (n + 1)]

            # persistent activations
            qrot, krot, aot = {}, {}, {}
            for ct in range(2):
                for ch in range(NCH):
                    qrot[ct, ch] = sg.tile([128, CHUNK], bf, tag=f"qr{ct}_{ch}",
                                           name=f"qr{ct}_{ch}")
                    aot[ct, ch] = sg.tile([128, CHUNK], bf, tag=f"ao{ct}_{ch}",
                                          name=f"ao{ct}_{ch}")
                for kc in range(3):
                    krot[ct, kc] = sg.tile([128, CHUNK], bf, tag=f"kr{ct}_{kc}",
                                           name=f"kr{ct}_{kc}")
            vaug = []
            for s in range(NSTC):
                vaug.append(sg.tile([128, 260], bf, tag=f"va{s}", name=f"va{s}"))

            pp = ctx.enter_context(tc.tile_pool(name="pp", bufs=8))
            rtmp = ctx.enter_context(tc.tile_pool(name="rtmp", bufs=4))
            dn = ctx.enter_context(tc.tile_pool(name="dn", bufs=4))
            stg = ctx.enter_context(tc.tile_pool(name="stg", bufs=4))
            ost = ctx.enter_context(tc.tile_pool(name="ost", bufs=4))

            # ---- phase B: K/V/Q projections + rope ----
            with tc.tile_pool(name="ppj", bufs=4, space="PSUM") as ppj, \
                 tc.tile_pool(name="ppw", bufs=2, space="PSUM") as ppw, \
                 tc.tile_pool(name="ppv", bufs=2, space="PSUM") as ppv:

                def proj_stage1(ct, var, bcol, dst, xtile, tile_cols, csl,
                                csb, ssb, tag):
                    """pm = x8 @ W8 (fp8 DoubleRow); dst = (pm+b)*C; u = (pm+b)*S~."""
                    w = csl.stop - csl.start
                    ps = ppj.tile([128, CHUNK], f32, tag="ps", name=f"pj_{tag}")
                    pm = ps[:, 0:w]
                    for p in range(2):
                        nc.tensor.matmul(pm, lhsT=wqk(var, ct, p),
                                         rhs=dr_rhs(xtile, tile_cols, p, csl),
                                         start=(p == 0), stop=(p == 1),
                                         perf_mode=DR)
                    nc.vector.scalar_tensor_tensor(
                        out=dst[:, :w], in0=pm, scalar=bias_sb[:, bcol:bcol + 1],
                        in1=csb, op0=ADD, op1=MULT)
                    u = rtmp.tile([128, CHUNK], bf, tag="u")
                    nc.vector.scalar_tensor_tensor(
                        out=u[:, :w], in0=pm, scalar=bias_sb[:, bcol:bcol + 1],
                        in1=ssb, op0=ADD, op1=MULT)
                    return dst, u, w, tag

                def proj_stage2(dst, u, w, tag):
                    """dst += P @ u (partition-pair swap via perm matmul)."""
                    psw = ppw.tile([128, CHUNK], f32, tag="psw",
                                   name=f"psw_{tag}")
                    nc.tensor.matmul(psw[:, 0:w], lhsT=perm_sb, rhs=u[:, :w],
                                     start=True, stop=True)
                    nc.vector.tensor_add(dst[:, :w], dst[:, :w], psw[:, 0:w])

                pend = []
                for ct in range(2):
                    for kc, w in enumerate(KCW):
                        csl = slice(CHUNK * kc, CHUNK * kc + w)
                        if len(pend) >= 2:
                            proj_stage2(*pend.pop(0))
                        pend.append(proj_stage1(
                            ct, 1, 2 + ct, krot[ct, kc], xtc8, NSC, csl,
                            trigk[:, 2 * NSC * ct + csl.start:
                                  2 * NSC * ct + csl.stop],
                            trigk[:, 2 * NSC * ct + NSC + csl.start:
                                  2 * NSC * ct + NSC + csl.stop],
                            f"k{ct}_{kc}"))
                for s in range(NSTC):
                    ssl = slice(128 * s, 128 * (s + 1))
                    pv = ppv.tile([128, 260], f32, tag="pv", name=f"pv{s}")
                    for kt in range(4):
                        nc.tensor.matmul(pv, lhsT=xtcv[:, NSC * kt + ssl.start:
                                                       NSC * kt + ssl.stop],
                                         rhs=wvall[:, 260 * kt:260 * (kt + 1)],
                                         start=(kt == 0), stop=(kt == 3))
                    nc.scalar.copy(out=vaug[s], in_=pv)
                    # denominator ones-column (0 at pad rows), bv is folded
                    # into the host-side bias via bv @ Wo
                    vo = vone_sb[:, s:s + 1]
                    vob = _bass.AP(tensor=vo.tensor, offset=vo.offset,
                                   ap=[vo.ap[0], [0, 4]])
                    nc.vector.tensor_copy(out=vaug[s][:, 64:260:65], in_=vob)
                    if pend and s % 2 == 1:
                        proj_stage2(*pend.pop(0))
                for j in range(NCH):
                    for ct in range(2):
                        csl = slice(CHUNK * j, CHUNK * (j + 1))
                        if len(pend) >= 2:
                            proj_stage2(*pend.pop(0))
                        pend.append(proj_stage1(
                            ct, 0, ct, qrot[ct, j], xt8, T, csl,
                            trigq[:, 2 * T * ct + csl.start:
                                  2 * T * ct + csl.stop],
                            trigq[:, 2 * T * ct + T + csl.start:
                                  2 * T * ct + T + csl.stop],
                            f"q{ct}_{j}"))
                while pend:
                    proj_stage2(*pend.pop(0))

            pps = ctx.enter_context(
                tc.tile_pool(name="pps", bufs=2, space="PSUM"))
            ppo = ctx.enter_context(tc.tile_pool(name="ppo", bufs=3, space="PSUM"))
            pprd = ctx.enter_context(tc.tile_pool(name="pprd", bufs=1, space="PSUM"))

            # ---- attention + output projection ----
            def atth(j):
                """scores/exp/mask/attv for both head pairs of chunk j, plus
                PSUM->SBUF staging and the reciprocal-denominator path.
                Returns per-hp staging handles for finish(j)."""
                tiles_j = _tiles_for_chunk(j)
                ret = []
                nstep = len(tiles_j)
                for hp in range(2):
                    ct = hp
                    po = [ppo.tile([65, CHUNK], f32, tag="po",
                                   name=f"po{j}_{hp}_{i}") for i in range(2)]

                    def stepA(si):
                        """scores pair + exp + band mask for s-step si."""
                        s, col0, crossing = tiles_j[si]
                        ksl = slice(128 * (s % 4), 128 * (s % 4) + 128)
                        ps = pps.tile([128, 2 * CHUNK], f32, tag="ps",
                                      name=f"ps{j}_{hp}_{s}")
                        for idx in range(2):
                            pb = 64 * idx
                            nc.tensor.matmul(
                                ps[:, CHUNK * idx + col0:CHUNK * (idx + 1)],
                                lhsT=krot[ct, s // 4][pb:pb + 64, ksl],
                                rhs=qrot[ct, j][pb:pb + 64, col0:],
                                start=True, stop=True)
                        pt = pp.tile([128, 2 * CHUNK], bf, tag="p",
                                     name=f"pt{j}_{hp}_{s}")
                        if col0:
                            for idx in range(2):
                                csl2 = slice(CHUNK * idx + col0, CHUNK * (idx + 1))
                                nc.scalar.activation(out=pt[:, csl2],
                                                     in_=ps[:, csl2],
                                                     func=EXP, scale=0.125)
                        else:
                            nc.scalar.activation(out=pt, in_=ps,
                                                 func=EXP, scale=0.125)
                        if crossing:
                            w2 = CHUNK - col0
                            base = pt[:, col0:col0 + w2]
                            pt2 = _bass.AP(tensor=base.tensor, offset=base.offset,
                                           ap=[base.ap[0], [CHUNK, 2], [1, w2]])
                            bmb = bm_slice(s, j, col0)
                            bm2 = _bass.AP(tensor=bmb.tensor, offset=bmb.offset,
                                           ap=[bmb.ap[0], [0, 2], [1, w2]])
                            nc.vector.tensor_mul(pt2, pt2, bm2)
                        return si, pt

                    def stepB(si, pt):
                        """att@v accumulation for s-step si."""
                        s, col0, crossing = tiles_j[si]
                        for idx in range(2):
                            hh = 2 * hp + idx
                            nc.tensor.matmul(
                                po[idx][:, col0:],
                                lhsT=vaug[s][:, 65 * hh:65 * hh + 65],
                                rhs=pt[:, CHUNK * idx + col0:CHUNK * (idx + 1)],
                                start=(si == 0), stop=(si == nstep - 1),
                                skip_group_check=True)

                    # lag-1 software pipeline: scores(s+1) issues before
                    # attv(s) so the PE never stalls on exp/mask latency
                    pend2 = []
                    for si in range(nstep):
                        a = stepA(si)
                        if pend2:
                            stepB(*pend2.pop(0))
                        pend2.append(a)
                    while pend2:
                        stepB(*pend2.pop(0))
                    # move PSUM results to SBUF (DMA cannot read PSUM)
                    oA = stg.tile([65, CHUNK], f32, tag="oA")
                    oB = stg.tile([65, CHUNK], f32, tag="oB")
                    nc.vector.tensor_copy(out=oA, in_=po[0])
                    nc.vector.tensor_copy(out=oB, in_=po[1])
                    shb = stg.tile([128, CHUNK], f32, tag="shb")
                    nc.sync.dma_start(out=shb[64:128, :], in_=oB[0:64, :])
                    dsb = dn.tile([128, 8], f32, tag="den")
                    nc.sync.dma_start(out=dsb[:, 0:4], in_=oA[64:65, :])
                    nc.sync.dma_start(out=dsb[:, 4:8], in_=oB[64:65, :])
                    rsb = dn.tile([128, 8], bf, tag="rden")
                    with nc.allow_low_precision(reason="bf16 softmax recip"):
                        nc.vector.reciprocal(rsb, dsb)
                    rdr = dn.tile([2, CHUNK], bf, tag="rdr")
                    nc.sync.dma_start(out=rdr[0:1, :], in_=rsb[:, 0:4])
                    nc.sync.dma_start(out=rdr[1:2, :], in_=rsb[:, 4:8])
                    ret.append((oA, shb, rdr))
                return ret

            def finish(j, data):
                """normalize chunk j (deferred past atth(j+1)) + out-projection."""
                for hp, (oA, shb, rdr) in enumerate(data):
                    prd = pprd.tile([128, CHUNK], f32, tag="prd")
                    nc.tensor.matmul(prd, lhsT=sel_sb, rhs=rdr,
                                     start=True, stop=True)
                    nc.vector.tensor_mul(aot[hp, j][0:64, :], oA[0:64, :],
                                         prd[0:64, :])
                    nc.vector.tensor_mul(aot[hp, j][64:128, :], shb[64:128, :],
                                         prd[64:128, :])
                for tt in range(4):
                    pout = ppo.tile([128, H], f32, tag="po", name=f"pout{j}_{tt}")
                    for ct2 in range(2):
                        nc.tensor.matmul(pout,
                                         lhsT=aot[ct2, j][:, 128 * tt:128 * (tt + 1)],
                                         rhs=woall[:, H * ct2:H * (ct2 + 1)],
                                         start=(ct2 == 0), stop=(ct2 == 1))
                    osb = ost.tile([128, H], bf, tag="ost")
                    nc.vector.tensor_copy(out=osb, in_=pout)
                    oeng = nc.sync if tt % 2 == 0 else nc.scalar
                    oeng.dma_start(
                        out=out_d[CHUNK * j + 128 * tt:CHUNK * j + 128 * (tt + 1), :],
                        in_=osb)

            prev = atth(0)
            for j in range(1, NCH):
                cur = atth(j)
                finish(j - 1, prev)
                prev = cur
            finish(NCH - 1, prev)

    nc.compile()
    _prog = nc
    return nc


def _host_inputs(x, Wq, bq, Wk, bk, Wv, bv, Wo, bo):
    """Build the 8 per-core input maps (packed mega-tensors, hardcoded shapes)."""
    x = np.asarray(x, np.float32)
    Wq, bq = np.asarray(Wq, np.float32), np.asarray(bq, np.float32)
    Wk, bk = np.asarray(Wk, np.float32), np.asarray(bk, np.float32)
    Wv, bv = np.asarray(Wv, np.float32), np.asarray(bv, np.float32)
    Wo = np.asarray(Wo, np.float32)

    def rowpack(a, cols):
        """[R*128, cols] -> [128, R*cols] row-tiles side by side."""
        r = a.shape[0] // 128
        return np.concatenate([a[128 * i:128 * (i + 1)] for i in range(r)], axis=1)

    xt8_all, xtc8_all, xtcv_all = [], [], []
    for b in range(B):
        xt = np.ascontiguousarray(x[b].T)            # (512, 2048)
        xtc = np.zeros((H, NSC), np.float32)
        xtc[:, :NSC_RAW] = xt[:, POS]
        xt8_all.append(rowpack(xt, T).astype(F8))
        xtc8_all.append(rowpack(xtc, NSC).astype(F8))
        xtcv_all.append(rowpack(xtc, NSC).astype(BF))
    vones = np.zeros((128, NSTC), np.float32)
    for s in range(NSTC):
        nvalid = max(0, min(128, NSC_RAW - 128 * s))
        vones[:nvalid, s] = 1.0

    # rope tables (match reference fp32 math)
    inv = (1.0 / (THETA ** (np.arange(0, H, 2, dtype=np.float32) / H))).astype(np.float32)
    tpos = np.arange(T, dtype=np.float32)
    ang = tpos[:, None] * inv[None, :]
    cosf = np.cos(ang).astype(np.float32).T     # (256, T)
    sinf = np.sin(ang).astype(np.float32).T

    def drpack(W):
        """[512, 128] -> [128, 256] DoubleRow stationary blocks (2 passes)."""
        out = np.zeros((2, 128, 2, 128), np.float32)
        for p in range(2):
            for i in range(2):
                out[p, :, i, :] = W[256 * p + 128 * i:256 * p + 128 * i + 128, :]
        return out.reshape(2, 128, 256)  # [pass][128, 256]

    per_g = []
    for g in range(2):
        cols = slice(CPG * g, CPG * (g + 1))
        wq_g, wk_g = Wq[:, cols], Wk[:, cols]
        wv_a = np.zeros((H, 260), np.float32)
        for hh in range(4):
            wv_a[:, 65 * hh:65 * hh + 64] = Wv[:, CPG * g + 64 * hh:CPG * g + 64 * (hh + 1)]
        # wqk8: [q|k] x [ct] x [pass] 256-col blocks
        blocks = []
        for W in (wq_g, wk_g):
            for ct in range(2):
                dp = drpack(W[:, 128 * ct:128 * (ct + 1)])
                blocks.extend([dp[0], dp[1]])
        wqk8 = np.concatenate(blocks, axis=1)       # [128, 2048]

        pr = slice(128 * g, 128 * (g + 1))
        cos_g = np.repeat(cosf[pr], 2, axis=0)      # C: repeat pairs
        sin_g = np.repeat(sinf[pr], 2, axis=0).copy()
        sin_g[1::2] *= -1.0                         # S~: minus on ODD rows
        cosk_g = np.zeros((CPG, NSC), np.float32)
        sink_g = np.zeros((CPG, NSC), np.float32)
        cosk_g[:, :NSC_RAW] = cos_g[:, POS]
        sink_g[:, :NSC_RAW] = sin_g[:, POS]
        # per-ct interleave: [Cq ct | Sq ct]... then [Ck ct | Sk ct]...
        cq, sq = rowpack(cos_g, T), rowpack(sin_g, T)        # [128, 2T] each
        ck, sk = rowpack(cosk_g, NSC), rowpack(sink_g, NSC)  # [128, 2NSC]
        trig = np.concatenate([cq[:, :T], sq[:, :T], cq[:, T:], sq[:, T:],
                               ck[:, :NSC], sk[:, :NSC],
                               ck[:, NSC:], sk[:, NSC:]], axis=1)
        biases = np.stack([
            bq[cols][:128], bq[cols][128:],
            bk[cols][:128], bk[cols][128:],
        ], axis=1).astype(np.float32)
        per_g.append(dict(
            wqk8=wqk8.astype(F8),
            wv=rowpack(wv_a, 260).astype(BF),
            wo=rowpack(Wo[cols, :], H).astype(BF),
            trig=trig.astype(F8), biases=biases,
        ))

    # causal band masks in compacted coords: valid iff POS[s] <= t
    spos = np.full(NSC, _INF, np.int64)
    spos[:NSC_RAW] = POS
    bmask = np.zeros((128, len(_CROSSINGS) * CHUNK), np.float32)
    for n, (i, j) in enumerate(_CROSSINGS):
        rows = spos[128 * i:128 * (i + 1)]
        tcols = np.arange(CHUNK * j, CHUNK * (j + 1))
        bmask[:, CHUNK * n:CHUNK * (n + 1)] = (rows[:, None] <= tcols[None, :])

    sel = np.zeros((2, 128), BF)
    sel[0, :64] = 1.0
    sel[1, 64:] = 1.0

    perm = np.zeros((128, 128), np.float32)
    perm[np.arange(128), np.arange(128) ^ 1] = 1.0

    shared = dict(bmask=bmask.astype(F8), sel=sel, perm=perm.astype(BF),
                  vones=vones)
    in_maps = []
    for c in range(NCORES):
        b, g = c // 2, c % 2
        m = dict(xt8=xt8_all[b], xtc8=xtc8_all[b], xtcv=xtcv_all[b], **shared)
        m.update(per_g[g])
        in_maps.append(m)
    return in_maps


def run(inputs, trace=False):
    """Build+run; returns BassKernelResults (per-core partials in .results)."""
    from concourse.bass_utils import run_bass_kernel_spmd
    nc = _build_program()
    in_maps = _host_inputs(**inputs)
    res = run_bass_kernel_spmd(nc, in_maps, list(range(NCORES)), trace=trace)
    return res


def assemble(results, Wv, bv, Wo, bo):
    """Sum per-core partials + host-folded bias (bv went through Wo)."""
    bo2 = (np.asarray(bo, np.float32)
           + np.asarray(bv, np.float32) @ np.asarray(Wo, np.float32))
    out = np.empty((B, T, H), np.float32)
    for b in range(B):
        out[b] = (results[2 * b]["out"].astype(np.float32)
                  + results[2 * b + 1]["out"].astype(np.float32)
                  + bo2[None, :])
    return out


def kernel(x, Wq, bq, Wk, bk, Wv, bv, Wo, bo):
    res = run(dict(x=x, Wq=Wq, bq=bq, Wk=Wk, bk=bk, Wv=Wv, bv=bv, Wo=Wo, bo=bo))
    return assemble(res.results, Wv, bv, Wo, bo)


# revision 70
# speedup vs baseline: 1.1063x; 1.0244x over previous
"""Trainium2 Bass kernel for masked causal attention with RoPE (mgdt column masking).

Reference computation (B=4, T=2048, H=512, heads=8, D=64):
  q/k/v = x @ W + b;  RoPE(q, k) over full hidden dim (pairs of adjacent channels);
  scores = q k^T / sqrt(D) with causal tril mask plus fully-masked key columns
  at {4, 7, 10, ...} (period 3); softmax; out = (att @ v) @ Wo + bo.

Sharding: 8 cores = data-parallel over batch (4) x tensor-parallel over head
groups (2 x 4 heads). Each core computes a [T, H] partial of its batch's
output projection (Wo row-sharded); host sums the pair of partials + bo.

Key structural choices (v2):
  - KEY COMPACTION: the 682 fully-masked key columns are removed on the host
    (x^T gathered at the 1366 unmasked positions, padded to 1408).
  - q/k are computed TRANSPOSED as qT[c, t] (c on partitions) in FP8 (e4m3)
    with DoubleRow matmuls (2 K-subtiles of 128 per instruction -> half the
    PE passes of bf16). Softmax washes out the fp8 noise on the scores path;
    V stays bf16 (its quantization error would land directly on the output).
  - RoPE without the swapped second projection: qrot = (q+b)*C + P@u,
    u = (q+b)*S~ where S~ is the sin table pre-swapped/sign-folded on the
    host and P is a 128x128 pair-swap permutation matmul (engines cannot
    partition-step; one K=128 F=512 matmul replaces two DR passes).
    bv is folded through Wo on the host (bo2 = bo + bv @ Wo), so the V
    projection has no bias pass; the denominator ones-column is restored
    with a tiny broadcast copy of per-tile valid flags.
  - scores are computed transposed (sT[s, t] per head, K=64 contraction, two
    heads packed in one [128, .] PSUM via PE row-tiling) so softmax's
    s-reduction and att@v's s-contraction keep s on partitions.
  - p = exp(sT/8) with no max subtraction (|scores/8| < ~2 by construction).
  - V is augmented per head with a ones column -> att@v emits the softmax
    denominator as row 64 of its PSUM output for free.
  - Normalization: denominators gathered (DMA), reciprocal'd in one [128, 8]
    DVE op, scattered back, broadcast over partitions with a K=2 selector
    matmul, applied with ONE [128, 512] multiply per (chunk, head-pair).
  - Inputs are loaded in consumption order, chunked, split across both HW
    DMA queues (sync + scalar) so the PE starts within ~1us. Trig tables and
    band masks ship as fp8; the output partial ships as bf16.
"""

import sys

if "/opt/trn_rl_repo" not in sys.path:
    sys.path.insert(0, "/opt/trn_rl_repo")

import numpy as np
import ml_dtypes

B, T, H, NH, D = 4, 2048, 512, 8, 64
THETA = 10000.0
PERIOD, RET_ORDER = 3, 2
NCORES = 8
CPG = H // 2          # 256 channels per head-group shard
CHUNK = 512           # t-chunk (one PSUM bank of fp32)
NCH = T // CHUNK      # 4 query chunks
BF = ml_dtypes.bfloat16
F8 = ml_dtypes.float8_e4m3

# --- compacted key geometry (host + builder share this) ---
_cm = np.ones(T, bool)
_cm[PERIOD + RET_ORDER - 1::PERIOD] = False
POS = np.where(_cm)[0]              # 1366 unmasked key positions
NSC_RAW = len(POS)                  # 1366
NSTC = (NSC_RAW + 127) // 128       # 11 s-tiles
NSC = NSTC * 128                    # 1408 padded
KCW = [CHUNK, CHUNK, NSC - 2 * CHUNK]   # k-projection chunk widths (512,512,384)

# per s-tile first/last valid original positions
_INF = 1 << 30
TILE_LO = [int(POS[128 * i]) if 128 * i < NSC_RAW else _INF for i in range(NSTC)]
TILE_HI = [int(POS[min(128 * i + 127, NSC_RAW - 1)]) if 128 * i < NSC_RAW else _INF
           for i in range(NSTC)]


def _tiles_for_chunk(j):
    """(i, col0, crossing) for each compact s-tile contributing to t-chunk j."""
    out = []
    for i in range(NSTC):
        lo, hi = TILE_LO[i], TILE_HI[i]
        if lo > CHUNK * j + CHUNK - 1:
            continue
        col0 = max(0, lo - CHUNK * j)
        crossing = hi > CHUNK * j  # some (row, col) pairs invalid -> needs mask
        out.append((i, col0, crossing))
    return out


_CROSSINGS = sorted({(i, j) for j in range(NCH)
                     for (i, c0, cr) in _tiles_for_chunk(j) if cr})

_prog = None


def _build_program():
    global _prog
    if _prog is not None:
        return _prog
    from contextlib import ExitStack
    import concourse.bacc as bacc
    import concourse.tile as tile
    import concourse.bass as _bass
    from concourse import mybir

    bf = mybir.dt.bfloat16
    f8 = mybir.dt.float8e4
    f32 = mybir.dt.float32
    EXP = mybir.ActivationFunctionType.Exp
    ADD = mybir.AluOpType.add
    MULT = mybir.AluOpType.mult
    DR = mybir.MatmulPerfMode.DoubleRow

    nc = bacc.Bacc("TRN2", target_bir_lowering=False, debug=False, num_devices=NCORES)

    def din(name, shape, dt):
        return nc.dram_tensor(name, shape, dt, kind="ExternalInput").ap()

    nm = len(_CROSSINGS)
    xt8_d = din("xt8", [128, 4 * T], f8)           # x^T 4 row-tiles side by side
    xtc8_d = din("xtc8", [128, 4 * NSC], f8)       # compacted x^T row-tiles
    xtcv_d = din("xtcv", [128, 4 * NSC], bf)       # bf16 copy for V projection
    wqk8_d = din("wqk8", [128, 2048], f8)          # q|k x ct x pass, 256-col blocks
    wv_d = din("wv", [128, 4 * 260], bf)
    wo_d = din("wo", [128, 2 * H], bf)
    # trig layout: [Cq ct0 | Sq~ ct0 | Cq ct1 | Sq~ ct1 | Ck ct0 | Sk~ ct0 |
    #               Ck ct1 | Sk~ ct1] so each ct's cos+sin are adjacent
    trig_d = din("trig", [128, 2 * (2 * T + 2 * NSC)], f8)
    bm_d = din("bmask", [128, nm * CHUNK], f8)
    sel_d = din("sel", [2, 128], bf)
    perm_d = din("perm", [128, 128], bf)           # partition pair-swap matrix
    vone_d = din("vones", [128, NSTC], f32)        # per s-tile valid-row flags
    bias_d = din("biases", [128, 4], f32)          # bq ct0|bq ct1|bk ct0|bk ct1
    out_d = nc.dram_tensor("out", [T, H], bf, kind="ExternalOutput").ap()

    TRIG_K = 4 * T

    with tile.TileContext(nc) as tc:
        with ExitStack() as ctx:
            sg = ctx.enter_context(tc.tile_pool(name="sg", bufs=1))

            def ld(name, dram, cols, dt=bf, nsplit=4, eng=None):
                eng = eng or nc.sync
                tl = sg.tile([128, cols], dt, tag=name, name=name)
                step = -(-cols // nsplit)
                for a in range(0, cols, step):
                    b = min(a + step, cols)
                    eng.dma_start(out=tl[:, a:b], in_=dram[:, a:b])
                return tl

            # ---- input loads in consumption order, split across 2 HW queues
            # sync queue: only the big projection inputs, first-use first;
            # fewer chunks = fewer 0.6us issue slots + less DMA-sem recycling
            wqk8 = sg.tile([128, 2048], f8, tag="wqk8", name="wqk8")
            nc.sync.dma_start(out=wqk8[:, 1024:2048], in_=wqk8_d[:, 1024:2048])
            xtc8 = ld("xtc8", xtc8_d, 4 * NSC, dt=f8, nsplit=2)
            trigk = sg.tile([128, 4 * NSC], f8, tag="trigk", name="trigk")
            for a in range(0, 4 * NSC, 2 * NSC):
                nc.sync.dma_start(out=trigk[:, a:a + 2 * NSC],
                                  in_=trig_d[:, TRIG_K + a:TRIG_K + a + 2 * NSC])
            nc.sync.dma_start(out=wqk8[:, 0:1024], in_=wqk8_d[:, 0:1024])
            xt8 = ld("xt8", xt8_d, 4 * T, dt=f8, nsplit=2)
            trigq = sg.tile([128, 4 * T], f8, tag="trigq", name="trigq")
            for a in range(0, 4 * T, 2 * T):
                nc.sync.dma_start(out=trigq[:, a:a + 2 * T],
                                  in_=trig_d[:, a:a + 2 * T])
            # scalar queue: small constants, then v projection + attention inputs
            bias_sb = sg.tile([128, 4], f32, tag="biases")
            nc.scalar.dma_start(out=bias_sb, in_=bias_d[:, :])
            perm_sb = sg.tile([128, 128], bf, tag="perm")
            nc.scalar.dma_start(out=perm_sb, in_=perm_d[:, :])
            sel_sb = sg.tile([2, 128], bf, tag="sel")
            nc.scalar.dma_start(out=sel_sb, in_=sel_d[:, :])
            vone_sb = sg.tile([128, NSTC], f32, tag="vones")
            nc.scalar.dma_start(out=vone_sb, in_=vone_d[:, :])
            xtcv = ld("xtcv", xtcv_d, 4 * NSC, dt=bf, nsplit=2, eng=nc.scalar)
            wvall = ld("wvall", wv_d, 4 * 260, dt=bf, nsplit=1, eng=nc.scalar)
            bmall = ld("bmall", bm_d, nm * CHUNK, dt=f8, nsplit=1, eng=nc.scalar)
            woall = ld("woall", wo_d, 2 * H, dt=bf, nsplit=1, eng=nc.scalar)

            def wqk(v, ct, p):
                """stationary [128, 2, 128] for variant v (0=q, 1=k), ct, pass p."""
                base = 1024 * v + 512 * ct + 256 * p
                t = wqk8[:, base:base + 256]
                return _bass.AP(tensor=t.tensor, offset=t.offset,
                                ap=[t.ap[0], [128, 2], [1, 128]])

            def dr_rhs(xtile, tile_cols, p, csl):
                """moving [128, 2, w] = row-tile pair (2p, 2p+1), cols csl."""
                t = xtile[:, tile_cols * 2 * p + csl.start:
                          tile_cols * 2 * p + csl.stop]
                return _bass.AP(tensor=t.tensor, offset=t.offset,
                                ap=[t.ap[0], [tile_cols, 2],
                                    [1, csl.stop - csl.start]])

            def bm_slice(i, j, c0):
                n = _CROSSINGS.index((i, j))
                return bmall[:, CHUNK * n + c0:CHUNK * (n + 1)]

            # persistent activations
            qrot, krot, aot = {}, {}, {}
            for ct in range(2):
                for ch in range(NCH):
                    qrot[ct, ch] = sg.tile([128, CHUNK], bf, tag=f"qr{ct}_{ch}",
                                           name=f"qr{ct}_{ch}")
                    aot[ct, ch] = sg.tile([128, CHUNK], bf, tag=f"ao{ct}_{ch}",
                                          name=f"ao{ct}_{ch}")
                for kc in range(3):
                    krot[ct, kc] = sg.tile([128, CHUNK], bf, tag=f"kr{ct}_{kc}",
                                           name=f"kr{ct}_{kc}")
            vaug = []
            for s in range(NSTC):
                vaug.append(sg.tile([128, 260], bf, tag=f"va{s}", name=f"va{s}"))

            pp = ctx.enter_context(tc.tile_pool(name="pp", bufs=8))
            rtmp = ctx.enter_context(tc.tile_pool(name="rtmp", bufs=4))
            dn = ctx.enter_context(tc.tile_pool(name="dn", bufs=4))
            stg = ctx.enter_context(tc.tile_pool(name="stg", bufs=4))
            ost = ctx.enter_context(tc.tile_pool(name="ost", bufs=4))

            # ---- phase B: K/V/Q projections + rope ----
            with tc.tile_pool(name="ppj", bufs=4, space="PSUM") as ppj, \
                 tc.tile_pool(name="ppw", bufs=2, space="PSUM") as ppw, \
                 tc.tile_pool(name="ppv", bufs=2, space="PSUM") as ppv:

                def proj_stage1(ct, var, bcol, dst, xtile, tile_cols, csl,
                                csb, ssb, tag):
                    """pm = x8 @ W8 (fp8 DoubleRow); dst = (pm+b)*C; u = (pm+b)*S~."""
                    w = csl.stop - csl.start
                    ps = ppj.tile([128, CHUNK], f32, tag="ps", name=f"pj_{tag}")
                    pm = ps[:, 0:w]
                    for p in range(2):
                        nc.tensor.matmul(pm, lhsT=wqk(var, ct, p),
                                         rhs=dr_rhs(xtile, tile_cols, p, csl),
                                         start=(p == 0), stop=(p == 1),
                                         perf_mode=DR)
                    nc.vector.scalar_tensor_tensor(
                        out=dst[:, :w], in0=pm, scalar=bias_sb[:, bcol:bcol + 1],
                        in1=csb, op0=ADD, op1=MULT)
                    u = rtmp.tile([128, CHUNK], bf, tag="u")
                    nc.vector.scalar_tensor_tensor(
                        out=u[:, :w], in0=pm, scalar=bias_sb[:, bcol:bcol + 1],
                        in1=ssb, op0=ADD, op1=MULT)
                    return dst, u, w, tag

                def proj_stage2(dst, u, w, tag):
                    """dst += P @ u (partition-pair swap via perm matmul)."""
                    psw = ppw.tile([128, CHUNK], f32, tag="psw",
                                   name=f"psw_{tag}")
                    nc.tensor.matmul(psw[:, 0:w], lhsT=perm_sb, rhs=u[:, :w],
                                     start=True, stop=True)
                    nc.vector.tensor_add(dst[:, :w], dst[:, :w], psw[:, 0:w])

                pend = []
                for ct in range(2):
                    for kc, w in enumerate(KCW):
                        csl = slice(CHUNK * kc, CHUNK * kc + w)
                        if len(pend) >= 2:
                            proj_stage2(*pend.pop(0))
                        pend.append(proj_stage1(
                            ct, 1, 2 + ct, krot[ct, kc], xtc8, NSC, csl,
                            trigk[:, 2 * NSC * ct + csl.start:
                                  2 * NSC * ct + csl.stop],
                            trigk[:, 2 * NSC * ct + NSC + csl.start:
                                  2 * NSC * ct + NSC + csl.stop],
                            f"k{ct}_{kc}"))
                # interleave V tiles (pure PE, no DVE) between Q projections so
                # the PE stays fed while the DVE rope chain catches up
                qcalls = [(j, ct) for j in range(NCH) for ct in range(2)]
                for s in range(NSTC):
                    ssl = slice(128 * s, 128 * (s + 1))
                    pv = ppv.tile([128, 260], f32, tag="pv", name=f"pv{s}")
                    for kt in range(4):
                        nc.tensor.matmul(pv, lhsT=xtcv[:, NSC * kt + ssl.start:
                                                       NSC * kt + ssl.stop],
                                         rhs=wvall[:, 260 * kt:260 * (kt + 1)],
                                         start=(kt == 0), stop=(kt == 3))
                    nc.scalar.copy(out=vaug[s], in_=pv)
                    # denominator ones-column (0 at pad rows), bv is folded
                    # into the host-side bias via bv @ Wo
                    vo = vone_sb[:, s:s + 1]
                    vob = _bass.AP(tensor=vo.tensor, offset=vo.offset,
                                   ap=[vo.ap[0], [0, 4]])
                    nc.vector.tensor_copy(out=vaug[s][:, 64:260:65], in_=vob)
                    if s >= 1 and qcalls:
                        if len(pend) >= 2:
                            proj_stage2(*pend.pop(0))
                        j, ct = qcalls.pop(0)
                        csl = slice(CHUNK * j, CHUNK * (j + 1))
                        pend.append(proj_stage1(
                            ct, 0, ct, qrot[ct, j], xt8, T, csl,
                            trigq[:, 2 * T * ct + csl.start:
                                  2 * T * ct + csl.stop],
                            trigq[:, 2 * T * ct + T + csl.start:
                                  2 * T * ct + T + csl.stop],
                            f"q{ct}_{j}"))
                    elif pend and s % 2 == 1:
                        proj_stage2(*pend.pop(0))
                while pend:
                    proj_stage2(*pend.pop(0))

            pps = ctx.enter_context(
                tc.tile_pool(name="pps", bufs=2, space="PSUM"))
            ppo = ctx.enter_context(tc.tile_pool(name="ppo", bufs=3, space="PSUM"))
            pprd = ctx.enter_context(tc.tile_pool(name="pprd", bufs=1, space="PSUM"))

            # ---- attention + output projection ----
            def atth(j):
                """scores/exp/mask/attv for both head pairs of chunk j, plus
                PSUM->SBUF staging and the reciprocal-denominator path.
                Returns per-hp staging handles for finish(j)."""
                tiles_j = _tiles_for_chunk(j)
                ret = []
                nstep = len(tiles_j)
                for hp in range(2):
                    ct = hp
                    po = [ppo.tile([65, CHUNK], f32, tag="po",
                                   name=f"po{j}_{hp}_{i}") for i in range(2)]

                    def stepA(si):
                        """scores pair + exp + band mask for s-step si."""
                        s, col0, crossing = tiles_j[si]
                        ksl = slice(128 * (s % 4), 128 * (s % 4) + 128)
                        ps = pps.tile([128, 2 * CHUNK], f32, tag="ps",
                                      name=f"ps{j}_{hp}_{s}")
                        for idx in range(2):
                            pb = 64 * idx
                            nc.tensor.matmul(
                                ps[:, CHUNK * idx + col0:CHUNK * (idx + 1)],
                                lhsT=krot[ct, s // 4][pb:pb + 64, ksl],
                                rhs=qrot[ct, j][pb:pb + 64, col0:],
                                start=True, stop=True)
                        pt = pp.tile([128, 2 * CHUNK], bf, tag="p",
                                     name=f"pt{j}_{hp}_{s}")
                        if col0:
                            for idx in range(2):
                                csl2 = slice(CHUNK * idx + col0, CHUNK * (idx + 1))
                                nc.scalar.activation(out=pt[:, csl2],
                                                     in_=ps[:, csl2],
                                                     func=EXP, scale=0.125)
                        else:
                            nc.scalar.activation(out=pt, in_=ps,
                                                 func=EXP, scale=0.125)
                        if crossing:
                            w2 = CHUNK - col0
                            base = pt[:, col0:col0 + w2]
                            pt2 = _bass.AP(tensor=base.tensor, offset=base.offset,
                                           ap=[base.ap[0], [CHUNK, 2], [1, w2]])
                            bmb = bm_slice(s, j, col0)
                            bm2 = _bass.AP(tensor=bmb.tensor, offset=bmb.offset,
                                           ap=[bmb.ap[0], [0, 2], [1, w2]])
                            nc.vector.tensor_mul(pt2, pt2, bm2)
                        return si, pt

                    def stepB(si, pt):
                        """att@v accumulation for s-step si."""
                        s, col0, crossing = tiles_j[si]
                        for idx in range(2):
                            hh = 2 * hp + idx
                            nc.tensor.matmul(
                                po[idx][:, col0:],
                                lhsT=vaug[s][:, 65 * hh:65 * hh + 65],
                                rhs=pt[:, CHUNK * idx + col0:CHUNK * (idx + 1)],
                                start=(si == 0), stop=(si == nstep - 1),
                                skip_group_check=True)

                    # lag-1 software pipeline: scores(s+1) issues before
                    # attv(s) so the PE never stalls on exp/mask latency
                    pend2 = []
                    for si in range(nstep):
                        a = stepA(si)
                        if pend2:
                            stepB(*pend2.pop(0))
                        pend2.append(a)
                    while pend2:
                        stepB(*pend2.pop(0))
                    # move PSUM results to SBUF (DMA cannot read PSUM)
                    oA = stg.tile([65, CHUNK], f32, tag="oA")
                    oB = stg.tile([65, CHUNK], f32, tag="oB")
                    nc.vector.tensor_copy(out=oA, in_=po[0])
                    nc.vector.tensor_copy(out=oB, in_=po[1])
                    shb = stg.tile([128, CHUNK], f32, tag="shb")
                    nc.sync.dma_start(out=shb[64:128, :], in_=oB[0:64, :])
                    dsb = dn.tile([128, 8], f32, tag="den")
                    nc.sync.dma_start(out=dsb[:, 0:4], in_=oA[64:65, :])
                    nc.sync.dma_start(out=dsb[:, 4:8], in_=oB[64:65, :])
                    rsb = dn.tile([128, 8], bf, tag="rden")
                    with nc.allow_low_precision(reason="bf16 softmax recip"):
                        nc.vector.reciprocal(rsb, dsb)
                    rdr = dn.tile([2, CHUNK], bf, tag="rdr")
                    nc.sync.dma_start(out=rdr[0:1, :], in_=rsb[:, 0:4])
                    nc.sync.dma_start(out=rdr[1:2, :], in_=rsb[:, 4:8])
                    ret.append((oA, shb, rdr))
                return ret

            def finish(j, data):
                """normalize chunk j (deferred past atth(j+1)) + out-projection."""
                for hp, (oA, shb, rdr) in enumerate(data):
                    prd = pprd.tile([128, CHUNK], f32, tag="prd")
                    nc.tensor.matmul(prd, lhsT=sel_sb, rhs=rdr,
                                     start=True, stop=True)
                    nc.vector.tensor_mul(aot[hp, j][0:64, :], oA[0:64, :],
                                         prd[0:64, :])
                    nc.vector.tensor_mul(aot[hp, j][64:128, :], shb[64:128, :],
                                         prd[64:128, :])
                for tt in range(4):
                    pout = ppo.tile([128, H], f32, tag="po", name=f"pout{j}_{tt}")
                    for ct2 in range(2):
                        nc.tensor.matmul(pout,
                                         lhsT=aot[ct2, j][:, 128 * tt:128 * (tt + 1)],
                                         rhs=woall[:, H * ct2:H * (ct2 + 1)],
                                         start=(ct2 == 0), stop=(ct2 == 1))
                    osb = ost.tile([128, H], bf, tag="ost")
                    nc.vector.tensor_copy(out=osb, in_=pout)
                    oeng = nc.sync if tt % 2 == 0 else nc.scalar
                    oeng.dma_start(
                        out=out_d[CHUNK * j + 128 * tt:CHUNK * j + 128 * (tt + 1), :],
                        in_=osb)

            prev = atth(0)
            for j in range(1, NCH):
                cur = atth(j)
                finish(j - 1, prev)
                prev = cur
            finish(NCH - 1, prev)

    nc.compile()
    _prog = nc
    return nc


def _host_inputs(x, Wq, bq, Wk, bk, Wv, bv, Wo, bo):
    """Build the 8 per-core input maps (packed mega-tensors, hardcoded shapes)."""
    x = np.asarray(x, np.float32)
    Wq, bq = np.asarray(Wq, np.float32), np.asarray(bq, np.float32)
    Wk, bk = np.asarray(Wk, np.float32), np.asarray(bk, np.float32)
    Wv, bv = np.asarray(Wv, np.float32), np.asarray(bv, np.float32)
    Wo = np.asarray(Wo, np.float32)

    def rowpack(a, cols):
        """[R*128, cols] -> [128, R*cols] row-tiles side by side."""
        r = a.shape[0] // 128
        return np.concatenate([a[128 * i:128 * (i + 1)] for i in range(r)], axis=1)

    xt8_all, xtc8_all, xtcv_all = [], [], []
    for b in range(B):
        xt = np.ascontiguousarray(x[b].T)            # (512, 2048)
        xtc = np.zeros((H, NSC), np.float32)
        xtc[:, :NSC_RAW] = xt[:, POS]
        xt8_all.append(rowpack(xt, T).astype(F8))
        xtc8_all.append(rowpack(xtc, NSC).astype(F8))
        xtcv_all.append(rowpack(xtc, NSC).astype(BF))
    vones = np.zeros((128, NSTC), np.float32)
    for s in range(NSTC):
        nvalid = max(0, min(128, NSC_RAW - 128 * s))
        vones[:nvalid, s] = 1.0

    # rope tables (match reference fp32 math)
    inv = (1.0 / (THETA ** (np.arange(0, H, 2, dtype=np.float32) / H))).astype(np.float32)
    tpos = np.arange(T, dtype=np.float32)
    ang = tpos[:, None] * inv[None, :]
    cosf = np.cos(ang).astype(np.float32).T     # (256, T)
    sinf = np.sin(ang).astype(np.float32).T

    def drpack(W):
        """[512, 128] -> [128, 256] DoubleRow stationary blocks (2 passes)."""
        out = np.zeros((2, 128, 2, 128), np.float32)
        for p in range(2):
            for i in range(2):
                out[p, :, i, :] = W[256 * p + 128 * i:256 * p + 128 * i + 128, :]
        return out.reshape(2, 128, 256)  # [pass][128, 256]

    per_g = []
    for g in range(2):
        cols = slice(CPG * g, CPG * (g + 1))
        wq_g, wk_g = Wq[:, cols], Wk[:, cols]
        wv_a = np.zeros((H, 260), np.float32)
        for hh in range(4):
            wv_a[:, 65 * hh:65 * hh + 64] = Wv[:, CPG * g + 64 * hh:CPG * g + 64 * (hh + 1)]
        # wqk8: [q|k] x [ct] x [pass] 256-col blocks
        blocks = []
        for W in (wq_g, wk_g):
            for ct in range(2):
                dp = drpack(W[:, 128 * ct:128 * (ct + 1)])
                blocks.extend([dp[0], dp[1]])
        wqk8 = np.concatenate(blocks, axis=1)       # [128, 2048]

        pr = slice(128 * g, 128 * (g + 1))
        cos_g = np.repeat(cosf[pr], 2, axis=0)      # C: repeat pairs
        sin_g = np.repeat(sinf[pr], 2, axis=0).copy()
        sin_g[1::2] *= -1.0                         # S~: minus on ODD rows
        cosk_g = np.zeros((CPG, NSC), np.float32)
        sink_g = np.zeros((CPG, NSC), np.float32)
        cosk_g[:, :NSC_RAW] = cos_g[:, POS]
        sink_g[:, :NSC_RAW] = sin_g[:, POS]
        # per-ct interleave: [Cq ct | Sq ct]... then [Ck ct | Sk ct]...
        cq, sq = rowpack(cos_g, T), rowpack(sin_g, T)        # [128, 2T] each
        ck, sk = rowpack(cosk_g, NSC), rowpack(sink_g, NSC)  # [128, 2NSC]
        trig = np.concatenate([cq[:, :T], sq[:, :T], cq[:, T:], sq[:, T:],
                               ck[:, :NSC], sk[:, :NSC],
                               ck[:, NSC:], sk[:, NSC:]], axis=1)
        biases = np.stack([
            bq[cols][:128], bq[cols][128:],
            bk[cols][:128], bk[cols][128:],
        ], axis=1).astype(np.float32)
        per_g.append(dict(
            wqk8=wqk8.astype(F8),
            wv=rowpack(wv_a, 260).astype(BF),
            wo=rowpack(Wo[cols, :], H).astype(BF),
            trig=trig.astype(F8), biases=biases,
        ))

    # causal band masks in compacted coords: valid iff POS[s] <= t
    spos = np.full(NSC, _INF, np.int64)
    spos[:NSC_RAW] = POS
    bmask = np.zeros((128, len(_CROSSINGS) * CHUNK), np.float32)
    for n, (i, j) in enumerate(_CROSSINGS):
        rows = spos[128 * i:128 * (i + 1)]
        tcols = np.arange(CHUNK * j, CHUNK * (j + 1))
        bmask[:, CHUNK * n:CHUNK * (n + 1)] = (rows[:, None] <= tcols[None, :])

    sel = np.zeros((2, 128), BF)
    sel[0, :64] = 1.0
    sel[1, 64:] = 1.0

    perm = np.zeros((128, 128), np.float32)
    perm[np.arange(128), np.arange(128) ^ 1] = 1.0

    shared = dict(bmask=bmask.astype(F8), sel=sel, perm=perm.astype(BF),
                  vones=vones)
    in_maps = []
    for c in range(NCORES):
        b, g = c // 2, c % 2
        m = dict(xt8=xt8_all[b], xtc8=xtc8_all[b], xtcv=xtcv_all[b], **shared)
        m.update(per_g[g])
        in_maps.append(m)
    return in_maps


def run(inputs, trace=False):
    """Build+run; returns BassKernelResults (per-core partials in .results)."""
    from concourse.bass_utils import run_bass_kernel_spmd
    nc = _build_program()
    in_maps = _host_inputs(**inputs)
    res = run_bass_kernel_spmd(nc, in_maps, list(range(NCORES)), trace=trace)
    return res


def assemble(results, Wv, bv, Wo, bo):
    """Sum per-core partials + host-folded bias (bv went through Wo)."""
    bo2 = (np.asarray(bo, np.float32)
           + np.asarray(bv, np.float32) @ np.asarray(Wo, np.float32))
    out = np.empty((B, T, H), np.float32)
    for b in range(B):
        out[b] = (results[2 * b]["out"].astype(np.float32)
                  + results[2 * b + 1]["out"].astype(np.float32)
                  + bo2[None, :])
    return out


def kernel(x, Wq, bq, Wk, bk, Wv, bv, Wo, bo):
    res = run(dict(x=x, Wq=Wq, bq=bq, Wk=Wk, bk=bk, Wv=Wv, bv=bv, Wo=Wo, bo=bo))
    return assemble(res.results, Wv, bv, Wo, bo)
